# revision 4
# baseline (speedup 1.0000x reference)
"""BotRGCN on 8 trn2 NeuronCores (SPMD, raw Bacc).

Nodes row-sharded 8 ways (12500/core, padded to 12800). Phase A
(768->8 projections etc.) fully sharded with PE transposes + bf16
matmuls. RGCN layers: AllGather bf16 node features -> shared gather
table; per-relation degree-sorted ELL plane gathers via indirect DMA
(int32 idx, 64B rows); DVE accumulate; per-node 1/cnt scale; DRAM
unpermute; dense matmuls in feature-major (^T) space.
"""
import sys
sys.path.insert(0, "/opt/trn_rl_repo")
from contextlib import ExitStack

import numpy as np
import ml_dtypes

from concourse import bacc, bass, mybir
from concourse import library_config
from concourse.bass_utils import run_bass_kernel_spmd

F32 = mybir.dt.float32
BF16 = mybir.dt.bfloat16
I32 = mybir.dt.int32
LRELU = mybir.ActivationFunctionType.Lrelu
ACOPY = mybir.ActivationFunctionType.Copy

N_CORES = 8
NUM_REL = 2
CBUF = 96          # gather piece size (idx columns per indirect DMA)


class Cfg:
    def __init__(self, shard_real, shard_pad, pieces):
        self.shard_real = shard_real
        self.shard_pad = shard_pad
        self.n_super = shard_pad // 512
        self.nt = shard_pad // 128
        self.c_unp = shard_pad // 128
        self.tabv = N_CORES * shard_pad
        # pieces[r] = list of (idx_col0, ck, adds) ; adds = [(agg_blk0, msg_blk0, nblk)]
        self.pieces = pieces
        self.ctot = (sum(ck for p in pieces for (_, ck, _) in [p[1]] for p in [p]) if False
                     else None)


def build_bass(cfg: Cfg):
    nc = bacc.Bacc("TRN2", debug=False)
    mmctx = ExitStack()
    SP = cfg.shard_pad
    NT = cfg.nt
    NS = cfg.n_super
    TABV = cfg.tabv
    CUNP = cfg.c_unp
    n_gcols = max(c0 + ck for r in range(NUM_REL) for (c0, ck, _) in cfg.pieces[r]) \
        if any(cfg.pieces) else 0
    CTOT = n_gcols + 2 * CUNP

    des_in = nc.declare_dram_parameter("des", [SP, 768], F32, isOutput=False)
    tw_in = nc.declare_dram_parameter("tweet", [SP, 768], F32, isOutput=False)
    ncat_in = nc.declare_dram_parameter("numcat", [SP, 16], F32, isOutput=False)
    idx_in = nc.declare_dram_parameter("idx", [128, CTOT], I32, isOutput=False)
    coef_in = nc.declare_dram_parameter("coefnm", [128, NUM_REL, NT], F32, isOutput=False)
    wd_in = nc.declare_dram_parameter("wd", [768, 32], BF16, isOutput=False)
    wt_in = nc.declare_dram_parameter("wt", [768, 32], BF16, isOutput=False)
    wnc_in = nc.declare_dram_parameter("wnc", [16, 32], BF16, isOutput=False)
    wsm_in = nc.declare_dram_parameter("wsm", [6, 33, 32], BF16, isOutput=False)
    bias_in = nc.declare_dram_parameter("biases", [32, 8], F32, isOutput=False)
    id128_in = nc.declare_dram_parameter("id128", [128, 128], F32, isOutput=False)
    id32_in = nc.declare_dram_parameter("id32", [32, 32], BF16, isOutput=False)
    out_ext = nc.declare_dram_parameter("out", [2, SP], F32, isOutput=True)

    shard_ag = nc.dram_tensor("shard_ag", [SP, 32], BF16)
    table = nc.dram_tensor("table", [TABV, 32], BF16, addr_space="Shared")
    scratch = nc.dram_tensor("scratch", [SP, 32], F32)

    live = []

    def sb(name, shape, dt):
        cm = nc.sbuf_tensor(name, shape, dt)
        t = cm.__enter__()
        live.append(cm)
        return t

    def psum_dt(name, shape, dt):
        cm = nc.psum_tensor(name, shape, dt)
        t = cm.__enter__()
        live.append(cm)
        return t

    def psum(name, shape):
        return psum_dt(name, shape, F32)

    sb_des = sb("sb_des", [128, 2, 4, 768], F32)
    sb_tw = sb("sb_tw", [128, 2, 4, 768], F32)
    sb_nc = sb("sb_nc", [128, 2, 4, 16], F32)
    sb_desT = sb("sb_desT", [128, 6, 512], BF16)
    sb_twT = sb("sb_twT", [128, 6, 512], BF16)
    sb_ncT = sb("sb_ncT", [16, 512], BF16)
    xT = sb("xT", [33, SP], BF16)
    aggT0 = sb("aggT0", [32, SP], BF16)
    aggT1 = sb("aggT1", [32, SP], BF16)
    agg = sb("agg", [128, NT * 32], F32)
    msgs2 = [sb("msgsA", [128, CBUF * 32], BF16), sb("msgsB", [128, CBUF * 32], BF16)]
    sb_shard = sb("sb_shard", [128, NT, 32], BF16)
    sb_idxg = [[sb(f"sb_idxg{r}_{pi}", [128, ck], I32)
                for pi, (c0, ck, _) in enumerate(cfg.pieces[r])] for r in range(NUM_REL)]
    sb_idxu = [sb(f"sb_idxu{r}", [128, CUNP], I32) for r in range(NUM_REL)]
    sb_coef = sb("sb_coef", [128, NUM_REL, NT], F32)
    sb_wd = sb("sb_wd", [128, 6, 32], BF16)
    sb_wt = sb("sb_wt", [128, 6, 32], BF16)
    sb_wnc = sb("sb_wnc", [16, 32], BF16)
    sb_wsm = sb("sb_wsm", [33, 6, 32], BF16)
    sb_bias = sb("sb_bias", [32, 8], F32)
    sb_id128 = sb("sb_id128", [128, 128], F32)
    sb_id32 = sb("sb_id32", [32, 32], BF16)
    sb_x3T = sb("sb_x3T", [33, 512], BF16)
    sb_lg = sb("sb_lg", [2, 2, 512], F32)

    pb = [psum(f"pb{i}", [128, 512]) for i in range(8)]
    pbx = pb[5][:, :].bitcast(BF16)

    plan = {"sync": [], "pe": [], "act": [], "dve": [], "gp": []}

    def op(engine, fn):
        plan[engine].append(fn)

    class Sem:
        def __init__(self, name):
            cm = nc.semaphore(name)
            self.h = cm.__enter__()
            live.append(cm)
            self.n = 0

        def inc(self, inst, k=1):
            # runtime half: attach the semaphore update (no counting here)
            inst.then_inc(self.h, k)

        def pinc(self, k=1):
            # plan-time half: advance the cumulative count
            self.n += k
            return self.n

    s_load = Sem("s_load")
    s_ld = [Sem("s_ld0"), Sem("s_ld1")]
    s_lr = Sem("s_lr")
    s_gq = [Sem(f"s_gq{i}") for i in range(8)]
    s_tp = Sem("s_tp")
    s_cp = Sem("s_cp")
    s_mm = Sem("s_mm")
    s_x1 = Sem("s_x1")
    s_gp = Sem("s_gp")
    s_cc = Sem("s_cc")
    s_dve = Sem("s_dve")
    s_sh = Sem("s_sh")

    def W(engine, sem, val):
        if val > 0:
            op(engine, lambda eng, s=sem, v=val: eng.wait_ge(s.h, v))

    # ---------------- constants ----------------
    def c_loads(eng):
        for r in range(NUM_REL):
            for pi, (c0, ck, _) in enumerate(cfg.pieces[r]):
                eng.dma_start(out=sb_idxg[r][pi][:], in_=idx_in[:, c0:c0 + ck]).then_inc(s_load.h, 16)
            u0 = n_gcols + r * CUNP
            eng.dma_start(out=sb_idxu[r][:], in_=idx_in[:, u0:u0 + CUNP]).then_inc(s_load.h, 16)
        eng.dma_start(out=sb_coef[:], in_=coef_in[:, :, :]).then_inc(s_load.h, 16)
        eng.dma_start(out=sb_wd[:], in_=wd_in.ap().rearrange("(c p) m -> p c m", p=128)).then_inc(s_load.h, 16)
        eng.dma_start(out=sb_wt[:], in_=wt_in.ap().rearrange("(c p) m -> p c m", p=128)).then_inc(s_load.h, 16)
        eng.dma_start(out=sb_wnc[:], in_=wnc_in[:, :]).then_inc(s_load.h, 16)
        eng.dma_start(out=sb_wsm[:], in_=wsm_in.ap().rearrange("c p m -> p c m")).then_inc(s_load.h, 16)
        eng.dma_start(out=sb_bias[:], in_=bias_in[:, :]).then_inc(s_load.h, 16)
        eng.dma_start(out=sb_id128[:], in_=id128_in[:, :]).then_inc(s_load.h, 16)
        eng.dma_start(out=sb_id32[:], in_=id32_in[:, :]).then_inc(s_load.h, 16)
    op("sync", c_loads)
    s_load.n += (8 + sum(len(p) for p in cfg.pieces) + NUM_REL) * 16
    NCONST = s_load.n

    op("gp", lambda eng: eng.load_library(library_config.mlp))

    def init_ones(eng):
        eng.memset(xT[32:33, :], 1.0)
        s_dve.inc(eng.memset(sb_x3T[32:33, :], 1.0))
    op("dve", init_ones)
    s_dve.pinc()
    NINIT = s_dve.n

    # =======================================================
    # Phase A
    # =======================================================
    for i in range(NS):
        buf = i % 2
        if i >= 2:
            W("sync", s_mm, 2 * (i - 1))
        for t in range(4):
            r0 = i * 512 + t * 128

            def ld(eng, r0=r0, buf=buf, t=t):
                eng.dma_start(out=sb_des[:, buf, t, :], in_=des_in[r0:r0 + 128, :]).then_inc(s_ld[buf].h, 16)
                eng.dma_start(out=sb_tw[:, buf, t, :], in_=tw_in[r0:r0 + 128, :]).then_inc(s_ld[buf].h, 16)
                eng.dma_start(out=sb_nc[:, buf, t, :], in_=ncat_in[r0:r0 + 128, :]).then_inc(s_ld[buf].h, 16)
            op("sync", ld)
            s_ld[buf].n += 48

        # ---- PE ----
        if i == 0:
            W("pe", s_load, NCONST)
        W("pe", s_ld[buf], 192 * (i // 2 + 1))
        if i >= 1:
            W("pe", s_cp, i * 5)          # ACT copy-rounds of i-1 done (psum WAR)
            W("pe", s_x1, 2 * i)          # act_x1(i-1) consumed pb7

        def pe_nc(eng, buf=buf):
            last = None
            for t in range(4):
                last = nc.tensor.transpose(out=pb[7][0:16, t * 128:(t + 1) * 128],
                                           in_=sb_nc[:, buf, t, :], identity=sb_id128[:])
            s_tp.inc(last)
        op("pe", pe_nc)
        s_tp.pinc()

        for (src, bank0, waitv) in ((sb_des, 0, i * 5 + 2), (sb_tw, 3, i * 5 + 4)):
            if bank0 == 3 and i >= 1:
                W("pe", s_sh, i)          # act_sh(i-1) consumed pb5-alias
            for rnd in range(2):
                if rnd == 1:
                    W("pe", s_cp, waitv)  # ACT copied round 0 of this tensor

                def pe_tp(eng, src=src, bank0=bank0, rnd=rnd, buf=buf):
                    last = None
                    for cc in range(3):
                        c = rnd * 3 + cc
                        for t in range(4):
                            last = nc.tensor.transpose(
                                out=pb[bank0 + cc][:, t * 128:(t + 1) * 128],
                                in_=src[:, buf, t, c * 128:(c + 1) * 128],
                                identity=sb_id128[:])
                    s_tp.inc(last)
                op("pe", pe_tp)
                s_tp.pinc()

        W("pe", s_cp, i * 5 + 5)

        def pe_mm(eng):
            for c in range(6):
                nc.tensor.matmul(pb[6][0:32, :], sb_wd[:, c, :], sb_desT[:, c, :],
                                 start=(c == 0), stop=False)
            for c in range(6):
                nc.tensor.matmul(pb[6][0:32, :], sb_wt[:, c, :], sb_twT[:, c, :],
                                 start=False, stop=False)
            last = nc.tensor.matmul(pb[6][0:32, :], sb_wnc[:, :], sb_ncT[:, :],
                                    start=False, stop=True)
            s_mm.inc(last)
        op("pe", pe_mm)
        s_mm.pinc()

        W("pe", s_x1, 2 * i + 1)
        if i == 0:
            W("pe", s_dve, NINIT)

        def pe_wi(eng, i=i):
            last = nc.tensor.matmul(pb[7][0:32, :], sb_wsm[:, 0, :],
                                    xT[0:33, i * 512:(i + 1) * 512], start=True, stop=True)
            s_mm.inc(last)
        op("pe", pe_wi)
        s_mm.pinc()

        W("pe", s_x1, 2 * i + 2)
        if i >= 1:
            W("pe", s_sh, i)              # act_sh(i-1) consumed pb0
        if i == NS - 1 and cfg.shard_real < SP:
            # pad memset is emitted after act_x1 in DVE order; predict its count
            W("pe", s_dve, NINIT + 1)

        def pe_x1t(eng, i=i):
            last = None
            for t in range(4):
                last = nc.tensor.transpose(
                    out=pbx[:, t * 32:(t + 1) * 32],
                    in_=xT[0:32, i * 512 + t * 128:i * 512 + (t + 1) * 128],
                    identity=sb_id32[:])
            s_tp.inc(last)
        op("pe", pe_x1t)
        s_tp.pinc()

        # ---- ACT ----
        W("act", s_tp, i * 6 + 1)
        op("act", lambda eng: s_cp.inc(eng.activation(out=sb_ncT[:, :], in_=pb[7][0:16, :],
                                                      func=ACOPY)))
        s_cp.pinc()
        k = 0
        for (dstT, bank0) in ((sb_desT, 0), (sb_twT, 3)):
            for rnd in range(2):
                k += 1
                W("act", s_tp, i * 6 + 1 + k)

                def act_cp(eng, dstT=dstT, bank0=bank0, rnd=rnd):
                    last = None
                    for cc in range(3):
                        c = rnd * 3 + cc
                        last = eng.activation(out=dstT[:, c, :], in_=pb[bank0 + cc][:, :],
                                              func=ACOPY)
                    s_cp.inc(last)
                op("act", act_cp)
                s_cp.pinc()

        W("act", s_mm, 2 * i + 1)
        op("act", lambda eng, i=i: s_lr.inc(eng.activation(
            out=xT[0:32, i * 512:(i + 1) * 512], in_=pb[6][0:32, :], func=ACOPY)))
        s_lr.pinc()
        W("dve", s_lr, s_lr.n)

        def act_x(eng, i=i):
            sl = xT[0:32, i * 512:(i + 1) * 512]
            s_x1.inc(nc.vector.scalar_tensor_tensor(
                out=sl, in0=sl, scalar=0.01, in1=sl,
                op0=mybir.AluOpType.mult, op1=mybir.AluOpType.max))
        op("dve", act_x)
        s_x1.pinc()
        W("act", s_mm, 2 * i + 2)
        op("act", lambda eng, i=i: s_lr.inc(eng.activation(
            out=xT[0:32, i * 512:(i + 1) * 512], in_=pb[7][0:32, :], func=ACOPY)))
        s_lr.pinc()
        W("dve", s_lr, s_lr.n)

        def act_x1(eng, i=i):
            sl = xT[0:32, i * 512:(i + 1) * 512]
            s_x1.inc(nc.vector.scalar_tensor_tensor(
                out=sl, in0=sl, scalar=0.01, in1=sl,
                op0=mybir.AluOpType.mult, op1=mybir.AluOpType.max))
        op("dve", act_x1)
        s_x1.pinc()
        if i == NS - 1 and cfg.shard_real < SP:
            W("dve", s_x1, 2 * NS)
            op("dve", lambda eng: s_dve.inc(eng.memset(xT[0:32, cfg.shard_real:SP], 0)))
            s_dve.pinc()
        W("act", s_tp, i * 6 + 6)

        def act_sh(eng, i=i):
            s_sh.inc(eng.activation(
                out=sb_shard[:, 4 * i:4 * i + 4, :].rearrange("p a b -> p (a b)"),
                in_=pbx[:, 0:128], func=ACOPY))
        op("act", act_sh)
        s_sh.pinc()

    # =======================================================
    # RGCN layers
    # =======================================================
    gq_rr = [0]

    def emit_layer(layer):
        W("gp", s_sh, s_sh.n)
        op("gp", lambda eng: s_gp.inc(
            eng.dma_start(out=shard_ag[:, :].rearrange("(t p) d -> p t d", p=128),
                          in_=sb_shard[:, :, :]), 16))
        s_gp.pinc(16)
        gp_shard_done = s_gp.n
        W("gp", s_gp, gp_shard_done)
        op("gp", lambda eng: s_cc.inc(eng.collective_compute(
            "AllGather", mybir.AluOpType.bypass,
            ins=[shard_ag[:, :]], outs=[table[:, :]],
            replica_groups=[list(range(N_CORES))])))
        s_cc.pinc()
        W("gp", s_cc, s_cc.n)

        for r in range(NUM_REL):
            W("dve", s_gp, s_gp.n)
            W("dve", s_tp, s_tp.n)   # prior relation's agg transposes done
            op("dve", lambda eng: s_dve.inc(eng.memset(agg[:, :], 0)))
            s_dve.pinc()
            D0 = s_dve.n
            npieces = len(cfg.pieces[r])
            for pi, (c0, ck, adds) in enumerate(cfg.pieces[r]):
                pbuf = pi % 2
                sq = (gq_rr[0]) % 8
                gq_rr[0] += 1
                # gather pi waits DVE finished consuming msgs[pbuf] (piece pi-2)
                W("gp", s_dve, D0 + pi - 1 if pi >= 2 else D0)

                def gth(eng, r=r, pi=pi, ck=ck, pbuf=pbuf, sq=sq):
                    for j in range(ck):
                        s_gq[sq].inc(eng.indirect_dma_start(
                            out=msgs2[pbuf][:, j * 32:(j + 1) * 32],
                            out_offset=None,
                            in_=table[:, :],
                            in_offset=bass.IndirectOffsetOnAxis(
                                ap=sb_idxg[r][pi][:, j:j + 1], axis=0),
                            bounds_check=TABV - 1,
                            oob_is_err=False,
                        ), 16)
                op("gp", gth)
                s_gq[sq].pinc(16 * ck)
                W("dve", s_gq[sq], s_gq[sq].n)

                def acc(eng, adds=adds, pbuf=pbuf):
                    last = None
                    for j, (ab, mb, nb) in enumerate(adds):
                        if j > 0:
                            eng.drain()
                        last = nc.vector.tensor_tensor(
                            out=agg[:, ab * 32:(ab + nb) * 32],
                            in0=agg[:, ab * 32:(ab + nb) * 32],
                            in1=msgs2[pbuf][:, mb * 32:(mb + nb) * 32],
                            op=mybir.AluOpType.add)
                    s_dve.inc(last)
                op("dve", acc)
                s_dve.pinc()
            # scale by 1/cnt (sorted order), per node-tile
            def scl(eng, r=r):
                eng.drain()
                last = None
                for t in range(NT):
                    last = nc.vector.tensor_tensor(
                        out=agg[:, t * 32:(t + 1) * 32],
                        in0=agg[:, t * 32:(t + 1) * 32],
                        in1=sb_coef[:, r, t:t + 1].to_broadcast([128, 32]),
                        op=mybir.AluOpType.mult)
                s_dve.inc(last)
            op("dve", scl)
            s_dve.pinc()
            # agg -> scratch (sorted) -> unperm gather back into agg (canonical)
            W("gp", s_dve, s_dve.n)
            op("gp", lambda eng: s_gp.inc(
                eng.dma_start(out=scratch[:, :].rearrange("(t p) d -> p t d", p=128),
                              in_=agg[:, :].rearrange("p (t d) -> p t d", d=32)), 16))
            s_gp.pinc(16)
            W("gp", s_gp, s_gp.n)
            def unp(eng, r=r):
                for j in range(CUNP):
                    s_gp.inc(eng.indirect_dma_start(
                        out=agg[:, j * 32:(j + 1) * 32],
                        out_offset=None,
                        in_=scratch[:, :],
                        in_offset=bass.IndirectOffsetOnAxis(ap=sb_idxu[r][:, j:j + 1], axis=0),
                        bounds_check=SP - 1,
                        oob_is_err=False,
                    ), 16)
            op("gp", unp)
            s_gp.pinc(16 * CUNP)
            # transpose agg (canonical node-major fp32) -> aggT (bf16 ^T)
            aggT = aggT0 if r == 0 else aggT1
            W("pe", s_gp, s_gp.n)
            C0 = s_cp.n
            T0 = s_tp.n
            for g in range(NT // 4):
                bank = pb[1 + (g % 2)]
                W("pe", s_cp, C0 + g - 1 if g >= 2 else C0)

                def pe_at(eng, g=g, bank=bank):
                    last = None
                    for t in range(4):
                        n = g * 4 + t
                        last = nc.tensor.transpose(out=bank[0:32, t * 128:(t + 1) * 128],
                                                   in_=agg[:, n * 32:(n + 1) * 32],
                                                   identity=sb_id128[:])
                    s_tp.inc(last)
                op("pe", pe_at)
                s_tp.pinc()
                W("act", s_tp, T0 + g + 1)

                def act_at(eng, g=g, bank=bank, aggT=aggT):
                    s_cp.inc(eng.activation(out=aggT[:, g * 512:(g + 1) * 512],
                                            in_=bank[0:32, :], func=ACOPY))
                op("act", act_at)
                s_cp.pinc()

        # dense tail
        W("pe", s_cp, s_cp.n)
        W("pe", s_x1, s_x1.n)
        X0 = s_x1.n
        for ch in range(NS):
            bank = pb[3 + (ch % 2)]
            if ch >= 2:
                W("pe", s_x1, X0 + ch - 1)

            def pe_tail(eng, ch=ch, bank=bank):
                sl = slice(ch * 512, (ch + 1) * 512)
                nc.tensor.matmul(bank[0:32, :], sb_wsm[:, 1, :], xT[0:33, sl],
                                 start=True, stop=False)
                nc.tensor.matmul(bank[0:32, :], sb_wsm[0:32, 2, :], aggT0[:, sl],
                                 start=False, stop=False)
                last = nc.tensor.matmul(bank[0:32, :], sb_wsm[0:32, 3, :], aggT1[:, sl],
                                        start=False, stop=True)
                s_mm.inc(last)
            op("pe", pe_tail)
            s_mm.pinc()
            W("act", s_mm, s_mm.n)

            def act_tail(eng, ch=ch, bank=bank):
                s_x1.inc(eng.activation(out=xT[0:32, ch * 512:(ch + 1) * 512],
                                        in_=bank[0:32, :], func=ACOPY))
            op("act", act_tail)
            s_x1.pinc()

        if layer == 1:
            S0 = s_sh.n
            X1 = X0
            for ch in range(NS):
                W("pe", s_x1, X1 + ch + 1)
                if ch >= 1:
                    W("pe", s_sh, S0 + ch)
                if ch == NS - 1 and cfg.shard_real < SP:
                    W("dve", s_x1, X1 + NS)
                    op("dve", lambda eng: s_dve.inc(eng.memset(xT[0:32, cfg.shard_real:SP], 0)))
                    s_dve.pinc()
                    W("pe", s_dve, s_dve.n)

                def pe_sh(eng, ch=ch):
                    last = None
                    for t in range(4):
                        last = nc.tensor.transpose(
                            out=pbx[:, t * 32:(t + 1) * 32],
                            in_=xT[0:32, ch * 512 + t * 128:ch * 512 + (t + 1) * 128],
                            identity=sb_id32[:])
                    s_tp.inc(last)
                op("pe", pe_sh)
                s_tp.pinc()
                W("act", s_tp, s_tp.n)
                if ch == 0:
                    W("act", s_gp, gp_shard_done)   # shard DMA of this layer done

                def act_sh2(eng, ch=ch):
                    s_sh.inc(eng.activation(
                        out=sb_shard[:, 4 * ch:4 * ch + 4, :].rearrange("p a b -> p (a b)"),
                        in_=pbx[:, 0:128], func=ACOPY))
                op("act", act_sh2)
                s_sh.pinc()

    emit_layer(1)
    emit_layer(2)

    # =======================================================
    # head
    # =======================================================
    W("pe", s_x1, s_x1.n)
    XH = s_x1.n
    GH = s_gp.n
    for ch in range(NS):
        bank = pb[3 + (ch % 2)]
        if ch >= 1:
            W("pe", s_x1, XH + 2 * ch)    # act_h2(ch-1) consumed bank... (2 acts/ch)

        def pe_h1(eng, ch=ch, bank=bank):
            s_mm.inc(nc.tensor.matmul(bank[0:32, :], sb_wsm[:, 4, :],
                                      xT[0:33, ch * 512:(ch + 1) * 512], start=True, stop=True))
        op("pe", pe_h1)
        s_mm.pinc()
        W("act", s_mm, s_mm.n)
        op("act", lambda eng, bank=bank: s_lr.inc(eng.activation(
            out=sb_x3T[0:32, :], in_=bank[0:32, :], func=ACOPY)))
        s_lr.pinc()
        W("dve", s_lr, s_lr.n)

        def act_h1(eng, ch=ch, bank=bank):
            s_x1.inc(nc.vector.scalar_tensor_tensor(
                out=sb_x3T[0:32, :], in0=sb_x3T[0:32, :], scalar=0.01,
                in1=sb_x3T[0:32, :], op0=mybir.AluOpType.mult, op1=mybir.AluOpType.max))
        op("dve", act_h1)
        s_x1.pinc()
        W("pe", s_x1, s_x1.n)

        def pe_h2(eng, ch=ch, bank=bank):
            s_mm.inc(nc.tensor.matmul(bank[0:2, :], sb_wsm[:, 5, 0:2],
                                      sb_x3T[0:33, :], start=True, stop=True))
        op("pe", pe_h2)
        s_mm.pinc()
        W("act", s_mm, s_mm.n)
        if ch >= 2:
            W("act", s_gp, GH + (ch - 1) * 16)

        def act_h2(eng, ch=ch, bank=bank):
            s_x1.inc(eng.activation(out=sb_lg[:, ch % 2, :], in_=bank[0:2, :],
                                    func=ACOPY))
        op("act", act_h2)
        s_x1.pinc()
        W("gp", s_x1, s_x1.n)

        def gp_out(eng, ch=ch):
            s_gp.inc(eng.dma_start(out=out_ext[:, ch * 512:(ch + 1) * 512],
                                   in_=sb_lg[:, ch % 2, :]), 16)
        op("gp", gp_out)
        s_gp.pinc(16)
    W("gp", s_gp, s_gp.n)

    with nc.Block() as block:
        @block.sync
        def _(eng):
            for f in plan["sync"]:
                f(eng)

        @block.tensor
        def _(eng):
            for f in plan["pe"]:
                f(eng)

        @block.scalar
        def _(eng):
            for f in plan["act"]:
                f(eng)

        @block.vector
        def _(eng):
            for f in plan["dve"]:
                f(eng)

        @block.gpsimd
        def _(eng):
            for f in plan["gp"]:
                f(eng)

    nc.compile()
    nc._live_refs = (live, mmctx)
    return nc


# =======================================================
# Host side
# =======================================================
def _build_structures(edge_index, edge_type, shard_real=12500, shard_pad=12800):
    SP = shard_pad
    src = edge_index[0].astype(np.int64)
    dst = edge_index[1].astype(np.int64)
    et = edge_type.astype(np.int64)
    owner = dst // shard_real
    ldst = dst % shard_real
    trow = (src // shard_real) * SP + (src % shard_real)

    per_core = []
    for c in range(N_CORES):
        rels = []
        for r in range(NUM_REL):
            sel = (owner == c) & (et == r)
            l = ldst[sel]
            t = trow[sel]
            dcnt = np.bincount(l, minlength=SP)
            perm = np.argsort(-dcnt, kind="stable")
            rank = np.empty(SP, dtype=np.int64)
            rank[perm] = np.arange(SP)
            order = np.argsort(rank[l], kind="stable")
            l_s, t_s = l[order], t[order]
            s_sorted = rank[l_s]
            if len(l_s):
                newgrp = np.r_[True, s_sorted[1:] != s_sorted[:-1]]
                gidx = np.cumsum(newgrp) - 1
                starts = np.flatnonzero(newgrp)
                kpos = np.arange(len(l_s)) - starts[gidx]
            else:
                kpos = np.zeros(0, dtype=np.int64)
            maxd = int(dcnt.max()) if len(l) else 0
            Lk = np.array([(dcnt > k).sum() for k in range(maxd)], dtype=np.int64)
            rels.append(dict(dcnt=dcnt, perm=perm, rank=rank, s=s_sorted, k=kpos,
                             t=t_s, maxd=maxd, Lk=Lk))
        per_core.append(rels)

    maxd_g = [max(per_core[c][r]["maxd"] for c in range(N_CORES)) for r in range(NUM_REL)]
    c_r = []
    for r in range(NUM_REL):
        cks = []
        for k in range(maxd_g[r]):
            m = 1
            for c in range(N_CORES):
                Lk = per_core[c][r]["Lk"]
                if k < len(Lk):
                    m = max(m, int(np.ceil(Lk[k] / 128)))
            cks.append(m)
        c_r.append(cks)

    # piece decomposition (shared across cores)
    pieces = []
    colbase_r = []
    gcol = 0
    for r in range(NUM_REL):
        colbase = []
        plist = []
        cur_c0 = gcol
        cur_ck = 0
        cur_adds = []
        for k, ck in enumerate(c_r[r]):
            colbase.append(gcol)
            # split plane k into runs that fit the piece
            off = 0
            while off < ck:
                room = CBUF - cur_ck
                if room == 0:
                    plist.append((cur_c0, cur_ck, cur_adds))
                    cur_c0, cur_ck, cur_adds = cur_c0 + CBUF, 0, []
                    room = CBUF
                take = min(room, ck - off)
                cur_adds.append((off, cur_ck, take))
                cur_ck += take
                off += take
            gcol += ck
        if cur_ck:
            plist.append((cur_c0, cur_ck, cur_adds))
        pieces.append(plist)
        colbase_r.append(np.array(colbase, dtype=np.int64))

    cfg = Cfg(shard_real, SP, pieces)
    return cfg, per_core, colbase_r, gcol


def _prep(inputs, shard_real=12500, shard_pad=12800):
    SP = shard_pad
    cfg, per_core, colbase_r, n_gcols = _build_structures(
        inputs["edge_index"], inputs["edge_type"], shard_real, shard_pad)
    NT = cfg.nt
    CUNP = cfg.c_unp
    CTOT = n_gcols + 2 * CUNP

    f32 = np.float32
    bf16 = ml_dtypes.bfloat16
    wd = np.zeros((768, 32), dtype=bf16); wd[:, 0:8] = inputs["Wd"].astype(bf16)
    wt = np.zeros((768, 32), dtype=bf16); wt[:, 8:16] = inputs["Wt"].astype(bf16)
    wnc = np.zeros((16, 32), dtype=bf16)
    wnc[0:6, 16:24] = inputs["Wn"].astype(bf16)
    wnc[6:9, 24:32] = inputs["Wc"].astype(bf16)
    bx = np.zeros(32, dtype=np.float32)
    bx[0:8] = inputs["bd"]; bx[8:16] = inputs["bt"]
    bx[16:24] = inputs["bn"]; bx[24:32] = inputs["bc"]
    wnc[9, :] = bx.astype(bf16)
    wsm = np.zeros((6, 33, 32), dtype=bf16)
    wsm[0, 0:32] = inputs["Wi"].astype(bf16)
    wsm[0, 32] = inputs["bi"].astype(bf16)
    wsm[1, 0:32] = inputs["Wroot"].astype(bf16)
    wsm[1, 32] = inputs["brgcn"].astype(bf16)
    wsm[2, 0:32] = inputs["Wrel"][0].astype(bf16)
    wsm[3, 0:32] = inputs["Wrel"][1].astype(bf16)
    wsm[4, 0:32] = inputs["Wo1"].astype(bf16)
    wsm[4, 32] = inputs["bo1"].astype(bf16)
    wsm[5, 0:32, 0:2] = inputs["Wo2"].astype(bf16)
    wsm[5, 32, 0:2] = inputs["bo2"].astype(bf16)
    biases = np.zeros((32, 8), dtype=f32)
    biases[0:8, 0] = inputs["bd"]; biases[8:16, 0] = inputs["bt"]
    biases[16:24, 0] = inputs["bn"]; biases[24:32, 0] = inputs["bc"]
    biases[:, 1] = inputs["bi"]; biases[:, 2] = inputs["brgcn"]
    biases[:, 3] = inputs["bo1"]; biases[0:2, 4] = inputs["bo2"]
    id128 = np.eye(128, dtype=f32)
    id32 = np.eye(32, dtype=bf16)

    in_maps = []
    for c in range(N_CORES):
        r0, r1 = c * shard_real, (c + 1) * shard_real
        des = np.zeros((SP, 768), dtype=f32); des[0:shard_real] = inputs["des"][r0:r1]
        tw = np.zeros((SP, 768), dtype=f32); tw[0:shard_real] = inputs["tweet"][r0:r1]
        ncat = np.zeros((SP, 16), dtype=f32)
        ncat[0:shard_real, 0:6] = inputs["num_prop"][r0:r1]
        ncat[0:shard_real, 6:9] = inputs["cat_prop"][r0:r1]
        ncat[0:shard_real, 9] = 1.0
        idx = np.full((128, CTOT), c * SP + SP - 1, dtype=np.int32)
        for r in range(NUM_REL):
            d = per_core[c][r]
            if len(d["s"]):
                jcol = d["s"] // 128
                p = d["s"] % 128
                cols = colbase_r[r][d["k"]] + jcol
                idx[p, cols] = d["t"].astype(np.int32)
        for r in range(NUM_REL):
            d = per_core[c][r]
            ucol = n_gcols + r * CUNP
            n = np.arange(SP)
            idx[n % 128, ucol + n // 128] = d["rank"][n].astype(np.int32)
        coefnm = np.zeros((128, NUM_REL, NT), dtype=f32)
        for r in range(NUM_REL):
            d = per_core[c][r]
            cv = (1.0 / np.maximum(d["dcnt"][d["perm"]], 1)).astype(f32)  # sorted order
            coefnm[:, r, :] = cv.reshape(NT, 128).T
        in_maps.append({
            "des": des, "tweet": tw, "numcat": ncat, "idx": idx, "coefnm": coefnm,
            "wd": wd, "wt": wt, "wnc": wnc, "wsm": wsm, "biases": biases,
            "id128": id128, "id32": id32,
        })
    return cfg, in_maps


_CACHE = {}


def kernel(**inputs):
    cfg, in_maps = _prep(inputs)
    key = tuple((c0, ck) for r in range(NUM_REL) for (c0, ck, _) in cfg.pieces[r])
    if key not in _CACHE:
        _CACHE[key] = build_bass(cfg)
    nc = _CACHE[key]
    res = run_bass_kernel_spmd(nc, in_maps, list(range(N_CORES)))
    outs = []
    for c in range(N_CORES):
        o = res.results[c]["out"]
        outs.append(o.T[0:cfg.shard_real])
    return np.ascontiguousarray(np.concatenate(outs, axis=0).astype(np.float32))



# revision 7
# speedup vs baseline: 1.1396x; 1.1396x over previous
"""BotRGCN on 8 trn2 NeuronCores (SPMD, raw Bacc).

Nodes row-sharded 8 ways (12500/core, padded to 12800). Phase A
(768->8 projections etc.) fully sharded with PE transposes + bf16
matmuls. RGCN layers: AllGather bf16 node features -> shared gather
table viewed as 256B groups of 4 node-rows; per-relation degree-sorted
ELL slot gathers via big SWDGE dma_gather instructions (int16 group
indices); DVE masked select-add (host-precomputed bf16 masks fold the
1/cnt mean scaling); unpermute via one dma_gather from a 256B-row
scratch; dense matmuls in feature-major (^T) space.
"""
import sys
sys.path.insert(0, "/opt/trn_rl_repo")
from contextlib import ExitStack

import numpy as np
import ml_dtypes

from concourse import bacc, bass, mybir
from concourse import library_config
from concourse.bass_utils import run_bass_kernel_spmd

F32 = mybir.dt.float32
BF16 = mybir.dt.bfloat16
I16 = mybir.dt.int16
LRELU = mybir.ActivationFunctionType.Lrelu
ACOPY = mybir.ActivationFunctionType.Copy

N_CORES = 8
NUM_REL = 2
CBUF = 96          # gather piece size (ELL columns per dma_gather)


class Cfg:
    def __init__(self, shard_real, shard_pad, pieces, n_gcols):
        self.shard_real = shard_real
        self.shard_pad = shard_pad
        self.n_super = shard_pad // 512
        self.nt = shard_pad // 128
        self.c_unp = shard_pad // 128
        self.tabv = N_CORES * shard_pad
        # pieces[r] = list of (col0, ck, adds) ; adds = [(agg_blk0, msg_col0, nblk)]
        self.pieces = pieces
        self.n_gcols = n_gcols


def build_bass(cfg: Cfg):
    nc = bacc.Bacc("TRN2", debug=False)
    mmctx = ExitStack()
    SP = cfg.shard_pad
    NT = cfg.nt
    NS = cfg.n_super
    TABV = cfg.tabv
    CUNP = cfg.c_unp
    NG = cfg.n_gcols

    des_in = nc.declare_dram_parameter("des", [SP, 768], F32, isOutput=False)
    tw_in = nc.declare_dram_parameter("tweet", [SP, 768], F32, isOutput=False)
    ncat_in = nc.declare_dram_parameter("numcat", [SP, 16], F32, isOutput=False)
    gconst_in = nc.declare_dram_parameter("gconst", [128, 12 * NG], I16, isOutput=False)
    uidx_in = nc.declare_dram_parameter("uidx", [128, NUM_REL, 8 * CUNP], I16, isOutput=False)
    wd_in = nc.declare_dram_parameter("wd", [768, 32], BF16, isOutput=False)
    wt_in = nc.declare_dram_parameter("wt", [768, 32], BF16, isOutput=False)
    wnc_in = nc.declare_dram_parameter("wnc", [16, 32], BF16, isOutput=False)
    wsm_in = nc.declare_dram_parameter("wsm", [6, 33, 32], BF16, isOutput=False)
    id128_in = nc.declare_dram_parameter("id128", [128, 128], F32, isOutput=False)
    id32_in = nc.declare_dram_parameter("id32", [32, 32], BF16, isOutput=False)
    out_ext = nc.declare_dram_parameter("out", [2, SP], F32, isOutput=True)

    shard_ag = nc.dram_tensor("shard_ag", [SP, 32], BF16)
    table = nc.dram_tensor("table", [TABV, 32], BF16, addr_space="Shared")
    scratch = [nc.dram_tensor(f"scratch{r}", [SP, 64], F32) for r in range(NUM_REL)]

    live = []

    def sb(name, shape, dt):
        cm = nc.sbuf_tensor(name, shape, dt)
        t = cm.__enter__()
        live.append(cm)
        return t

    def psum(name, shape):
        cm = nc.psum_tensor(name, shape, F32)
        t = cm.__enter__()
        live.append(cm)
        return t

    sb_des = sb("sb_des", [128, 2, 4, 768], F32)
    sb_tw = sb("sb_tw", [128, 2, 4, 768], F32)
    sb_nc = sb("sb_nc", [128, 2, 4, 16], F32)
    sb_desT = sb("sb_desT", [128, 6, 512], BF16)
    sb_twT = sb("sb_twT", [128, 6, 512], BF16)
    sb_ncT = sb("sb_ncT", [16, 512], BF16)
    xT = sb("xT", [33, SP], BF16)
    aggT0 = sb("aggT0", [32, SP], BF16)
    aggT1 = sb("aggT1", [32, SP], BF16)
    agg = sb("agg", [128, NT * 32], F32)
    # gather piece buffers alias the phase-A des/tweet staging (consumed by then)
    msgs2 = [sb_des[:, :, :, :].bitcast(BF16).rearrange("p a b c -> p (a b c)")
             .rearrange("p (a b) -> p a b", b=128),
             sb_tw[:, :, :, :].bitcast(BF16).rearrange("p a b c -> p (a b c)")
             .rearrange("p (a b) -> p a b", b=128)]
    tmp2 = [sb(f"tmp{i}", [128, CBUF, 32], BF16) for i in range(2)]
    sb_gc = sb("sb_gc", [128, 2, 12 * CBUF], I16)
    sb_uidx = sb("sb_uidx", [128, NUM_REL, 8 * CUNP], I16)
    unp = sb("unp", [128, CUNP, 64], F32)
    sb_shard = sb("sb_shard", [128, NT, 32], BF16)
    sb_wd = sb("sb_wd", [128, 6, 32], BF16)
    sb_wt = sb("sb_wt", [128, 6, 32], BF16)
    sb_wnc = sb("sb_wnc", [16, 32], BF16)
    sb_wsm = sb("sb_wsm", [33, 6, 32], BF16)
    sb_id128 = sb("sb_id128", [128, 128], F32)
    sb_id32 = sb("sb_id32", [32, 32], BF16)
    sb_x3T = sb("sb_x3T", [33, 512], BF16)
    sb_lg = sb("sb_lg", [2, 2, 512], F32)

    pb = [psum(f"pb{i}", [128, 512]) for i in range(8)]
    pbx = pb[5][:, :].bitcast(BF16)

    table4 = table.ap().rearrange("(g f) d -> g (f d)", f=4)   # [TABV/4, 128]

    plan = {"sync": [], "pe": [], "act": [], "dve": [], "gp": []}

    def op(engine, fn):
        plan[engine].append(fn)

    class Sem:
        def __init__(self, name):
            cm = nc.semaphore(name)
            self.h = cm.__enter__()
            live.append(cm)
            self.n = 0

        def inc(self, inst, k=1):
            inst.then_inc(self.h, k)

        def pinc(self, k=1):
            self.n += k
            return self.n

    s_load = Sem("s_load")
    s_ld = [Sem("s_ld0"), Sem("s_ld1")]
    s_lr = Sem("s_lr")
    s_gq = Sem("s_gq")      # SWDGE gathers (dedicated)
    s_gc = Sem("s_gc")      # gconst piece loads
    s_tp = Sem("s_tp")
    s_cp = Sem("s_cp")
    s_mm = Sem("s_mm")
    s_x1 = Sem("s_x1")
    s_gp = Sem("s_gp")      # gp-engine bulk DMAs (SWDGE)
    s_cc = Sem("s_cc")
    s_dve = Sem("s_dve")
    s_sh = Sem("s_sh")

    def W(engine, sem, val):
        if val > 0:
            op(engine, lambda eng, s=sem, v=val: eng.wait_ge(s.h, v))

    # ---------------- constants ----------------
    def c_loads(eng):
        eng.dma_start(out=sb_uidx[:], in_=uidx_in[:, :, :]).then_inc(s_load.h, 16)
        eng.dma_start(out=sb_wd[:], in_=wd_in.ap().rearrange("(c p) m -> p c m", p=128)).then_inc(s_load.h, 16)
        eng.dma_start(out=sb_wt[:], in_=wt_in.ap().rearrange("(c p) m -> p c m", p=128)).then_inc(s_load.h, 16)
        eng.dma_start(out=sb_wnc[:], in_=wnc_in[:, :]).then_inc(s_load.h, 16)
        eng.dma_start(out=sb_wsm[:], in_=wsm_in.ap().rearrange("c p m -> p c m")).then_inc(s_load.h, 16)
        eng.dma_start(out=sb_id128[:], in_=id128_in[:, :]).then_inc(s_load.h, 16)
        eng.dma_start(out=sb_id32[:], in_=id32_in[:, :]).then_inc(s_load.h, 16)
    op("sync", c_loads)
    s_load.n += 7 * 16
    NCONST = s_load.n

    op("gp", lambda eng: eng.load_library(library_config.mlp))

    def init_ones(eng):
        eng.memset(xT[32:33, :], 1.0)
        s_dve.inc(eng.memset(sb_x3T[32:33, :], 1.0))
    op("dve", init_ones)
    s_dve.pinc()
    NINIT = s_dve.n

    # =======================================================
    # Phase A
    # =======================================================
    for i in range(NS):
        buf = i % 2
        if i >= 2:
            W("sync", s_mm, 2 * (i - 1))

        def ld(eng, i=i, buf=buf):
            r0 = i * 512
            eng.dma_start(out=sb_des[:, buf, :, :],
                          in_=des_in[r0:r0 + 512, :].rearrange("(t p) c -> p t c", p=128)
                          ).then_inc(s_ld[buf].h, 16)
            eng.dma_start(out=sb_tw[:, buf, :, :],
                          in_=tw_in[r0:r0 + 512, :].rearrange("(t p) c -> p t c", p=128)
                          ).then_inc(s_ld[buf].h, 16)
            eng.dma_start(out=sb_nc[:, buf, :, :],
                          in_=ncat_in[r0:r0 + 512, :].rearrange("(t p) c -> p t c", p=128)
                          ).then_inc(s_ld[buf].h, 16)
        op("sync", ld)
        s_ld[buf].n += 48

        # ---- PE ----
        if i == 0:
            W("pe", s_load, NCONST)
        W("pe", s_ld[buf], 48 * (i // 2 + 1))
        if i >= 1:
            W("pe", s_cp, i * 5)          # ACT copy-rounds of i-1 done (psum WAR)
            W("pe", s_x1, 2 * i)          # act_x1(i-1) consumed pb7

        def pe_nc(eng, buf=buf):
            last = None
            for t in range(4):
                last = nc.tensor.transpose(out=pb[7][0:16, t * 128:(t + 1) * 128],
                                           in_=sb_nc[:, buf, t, :], identity=sb_id128[:])
            s_tp.inc(last)
        op("pe", pe_nc)
        s_tp.pinc()

        for (src, bank0, waitv) in ((sb_des, 0, i * 5 + 2), (sb_tw, 3, i * 5 + 4)):
            if bank0 == 3 and i >= 1:
                W("pe", s_sh, i)          # act_sh(i-1) consumed pb5-alias
            for rnd in range(2):
                if rnd == 1:
                    W("pe", s_cp, waitv)  # ACT copied round 0 of this tensor

                def pe_tp(eng, src=src, bank0=bank0, rnd=rnd, buf=buf):
                    last = None
                    for cc in range(3):
                        c = rnd * 3 + cc
                        for t in range(4):
                            last = nc.tensor.transpose(
                                out=pb[bank0 + cc][:, t * 128:(t + 1) * 128],
                                in_=src[:, buf, t, c * 128:(c + 1) * 128],
                                identity=sb_id128[:])
                    s_tp.inc(last)
                op("pe", pe_tp)
                s_tp.pinc()

        W("pe", s_cp, i * 5 + 5)

        def pe_mm(eng):
            for c in range(6):
                nc.tensor.matmul(pb[6][0:32, :], sb_wd[:, c, :], sb_desT[:, c, :],
                                 start=(c == 0), stop=False)
            for c in range(6):
                nc.tensor.matmul(pb[6][0:32, :], sb_wt[:, c, :], sb_twT[:, c, :],
                                 start=False, stop=False)
            last = nc.tensor.matmul(pb[6][0:32, :], sb_wnc[:, :], sb_ncT[:, :],
                                    start=False, stop=True)
            s_mm.inc(last)
        op("pe", pe_mm)
        s_mm.pinc()

        W("pe", s_x1, 2 * i + 1)
        if i == 0:
            W("pe", s_dve, NINIT)

        def pe_wi(eng, i=i):
            last = nc.tensor.matmul(pb[7][0:32, :], sb_wsm[:, 0, :],
                                    xT[0:33, i * 512:(i + 1) * 512], start=True, stop=True)
            s_mm.inc(last)
        op("pe", pe_wi)
        s_mm.pinc()

        W("pe", s_x1, 2 * i + 2)
        if i >= 1:
            W("pe", s_sh, i)              # act_sh(i-1) consumed pb0
        if i == NS - 1 and cfg.shard_real < SP:
            W("pe", s_dve, NINIT + 1)

        def pe_x1t(eng, i=i):
            last = None
            for t in range(4):
                last = nc.tensor.transpose(
                    out=pbx[:, t * 32:(t + 1) * 32],
                    in_=xT[0:32, i * 512 + t * 128:i * 512 + (t + 1) * 128],
                    identity=sb_id32[:])
            s_tp.inc(last)
        op("pe", pe_x1t)
        s_tp.pinc()

        # ---- ACT ----
        W("act", s_tp, i * 6 + 1)
        op("act", lambda eng: s_cp.inc(eng.activation(out=sb_ncT[:, :], in_=pb[7][0:16, :],
                                                      func=ACOPY)))
        s_cp.pinc()
        k = 0
        for (dstT, bank0) in ((sb_desT, 0), (sb_twT, 3)):
            for rnd in range(2):
                k += 1
                W("act", s_tp, i * 6 + 1 + k)

                def act_cp(eng, dstT=dstT, bank0=bank0, rnd=rnd):
                    last = None
                    for cc in range(3):
                        c = rnd * 3 + cc
                        last = eng.activation(out=dstT[:, c, :], in_=pb[bank0 + cc][:, :],
                                              func=ACOPY)
                    s_cp.inc(last)
                op("act", act_cp)
                s_cp.pinc()

        W("act", s_mm, 2 * i + 1)
        op("act", lambda eng, i=i: s_lr.inc(eng.activation(
            out=xT[0:32, i * 512:(i + 1) * 512], in_=pb[6][0:32, :], func=ACOPY)))
        s_lr.pinc()
        W("dve", s_lr, s_lr.n)

        def act_x(eng, i=i):
            sl = xT[0:32, i * 512:(i + 1) * 512]
            s_x1.inc(nc.vector.scalar_tensor_tensor(
                out=sl, in0=sl, scalar=0.01, in1=sl,
                op0=mybir.AluOpType.mult, op1=mybir.AluOpType.max))
        op("dve", act_x)
        s_x1.pinc()
        W("act", s_mm, 2 * i + 2)
        op("act", lambda eng, i=i: s_lr.inc(eng.activation(
            out=xT[0:32, i * 512:(i + 1) * 512], in_=pb[7][0:32, :], func=ACOPY)))
        s_lr.pinc()
        W("dve", s_lr, s_lr.n)

        def act_x1(eng, i=i):
            sl = xT[0:32, i * 512:(i + 1) * 512]
            s_x1.inc(nc.vector.scalar_tensor_tensor(
                out=sl, in0=sl, scalar=0.01, in1=sl,
                op0=mybir.AluOpType.mult, op1=mybir.AluOpType.max))
        op("dve", act_x1)
        s_x1.pinc()
        if i == NS - 1 and cfg.shard_real < SP:
            W("dve", s_x1, 2 * NS)
            op("dve", lambda eng: s_dve.inc(eng.memset(xT[0:32, cfg.shard_real:SP], 0)))
            s_dve.pinc()
        W("act", s_tp, i * 6 + 6)

        def act_sh(eng, i=i):
            s_sh.inc(eng.activation(
                out=sb_shard[:, 4 * i:4 * i + 4, :].rearrange("p a b -> p (a b)"),
                in_=pbx[:, 0:128], func=ACOPY))
        op("act", act_sh)
        s_sh.pinc()

    # =======================================================
    # RGCN layers
    # =======================================================
    glob = {"gpi": 0}
    piece_dve_after = []   # s_dve count after global piece j's select-adds

    def emit_layer(layer):
        W("gp", s_sh, s_sh.n)
        op("gp", lambda eng: s_gp.inc(
            eng.dma_start(out=shard_ag[:, :].rearrange("(t p) d -> p t d", p=128),
                          in_=sb_shard[:, :, :]), 16))
        s_gp.pinc(16)
        gp_shard_done = s_gp.n
        W("gp", s_gp, gp_shard_done)
        op("gp", lambda eng: s_cc.inc(eng.collective_compute(
            "AllGather", mybir.AluOpType.bypass,
            ins=[shard_ag[:, :]], outs=[table[:, :]],
            replica_groups=[list(range(N_CORES))])))
        s_cc.pinc()
        CCN = s_cc.n

        for r in range(NUM_REL):
            # memset agg; waits prior relation's scratch write (agg WAR)
            W("dve", s_gp, s_gp.n)
            op("dve", lambda eng: s_dve.inc(eng.memset(agg[:, :], 0)))
            s_dve.pinc()

            for pi, (c0, ck, adds) in enumerate(cfg.pieces[r]):
                j = glob["gpi"]
                glob["gpi"] += 1
                pbuf = j % 2
                # stream this piece's idx+cmask block
                if j >= 2:
                    W("sync", s_dve, piece_dve_after[j - 2])
                op("sync", lambda eng, c0=c0, ck=ck, pbuf=pbuf: s_gc.inc(
                    eng.dma_start(out=sb_gc[:, pbuf, 0:12 * ck],
                                  in_=gconst_in[:, 12 * c0:12 * (c0 + ck)]), 16))
                s_gc.pinc(16)
                gcv = s_gc.n
                # gather on gp
                W("gp", s_gc, gcv)
                if pi == 0:
                    W("gp", s_cc, CCN)
                if j >= 2:
                    W("gp", s_dve, piece_dve_after[j - 2])   # msgs/gc WAR

                def gth(eng, ck=ck, pbuf=pbuf):
                    s_gq.inc(eng.dma_gather(
                        out_ap=msgs2[pbuf][:, 0:ck, :],
                        in_ap=table4,
                        idxs_ap=sb_gc[:, pbuf, 0:8 * ck],
                        num_idxs=128 * ck,
                        num_idxs_reg=128 * ck,
                        elem_size=128,
                        single_packet=False,
                    ), 16)
                op("gp", gth)
                s_gq.pinc(16)
                GQ = s_gq.n
                # DVE select-adds
                W("dve", s_gq, GQ)

                def sel(eng, ck=ck, adds=adds, pbuf=pbuf):
                    cmbase = 8 * ck
                    gcb = sb_gc[:, pbuf, :].bitcast(BF16)   # [128, 12*CBUF]
                    last = None
                    for b in range(4):
                        tb = tmp2[b % 2]
                        if b > 0:
                            eng.drain()
                        nc.vector.tensor_tensor(
                            out=tb[:, 0:ck, :],
                            in0=msgs2[pbuf][:, 0:ck, 32 * b:32 * b + 32],
                            in1=gcb[:, cmbase + b * ck:cmbase + (b + 1) * ck]
                                .to_broadcast([128, ck, 32]),
                            op=mybir.AluOpType.mult)
                        for (ab, mb, nb) in adds:
                            eng.drain()
                            last = nc.vector.tensor_tensor(
                                out=agg[:, ab * 32:(ab + nb) * 32],
                                in0=agg[:, ab * 32:(ab + nb) * 32],
                                in1=tb[:, mb:mb + nb, :].rearrange("p a b -> p (a b)"),
                                op=mybir.AluOpType.add)
                    s_dve.inc(last)
                op("dve", sel)
                s_dve.pinc()
                piece_dve_after.append(s_dve.n)

            # relation done: agg (sorted order) -> scratch (256B rows)
            W("gp", s_dve, s_dve.n)
            op("gp", lambda eng, r=r: s_gp.inc(
                eng.dma_start(out=scratch[r][:, 0:32].rearrange("(t p) d -> p t d", p=128),
                              in_=agg[:, :].rearrange("p (t d) -> p t d", d=32)), 16))
            s_gp.pinc(16)
            W("gp", s_gp, s_gp.n)
            W("gp", s_tp, s_tp.n)   # prior relation's unp transposes done (unp WAR)

            def unp_g(eng, r=r):
                s_gq.inc(eng.dma_gather(
                    out_ap=unp[:, :, :],
                    in_ap=scratch[r].ap(),
                    idxs_ap=sb_uidx[:, r, :],
                    num_idxs=SP,
                    num_idxs_reg=SP,
                    elem_size=64,
                    single_packet=False,
                ), 16)
            op("gp", unp_g)
            s_gq.pinc(16)
            UNPQ = s_gq.n

            # transpose unp (canonical node-major fp32) -> aggT (bf16 ^T)
            aggT = aggT0 if r == 0 else aggT1
            W("pe", s_gq, UNPQ)
            C0 = s_cp.n
            T0 = s_tp.n
            for g in range(NT // 4):
                bank = pb[1 + (g % 2)]
                W("pe", s_cp, C0 + g - 1 if g >= 2 else C0)

                def pe_at(eng, g=g, bank=bank):
                    last = None
                    for t in range(4):
                        n = g * 4 + t
                        last = nc.tensor.transpose(out=bank[0:32, t * 128:(t + 1) * 128],
                                                   in_=unp[:, n, 0:32],
                                                   identity=sb_id128[:])
                    s_tp.inc(last)
                op("pe", pe_at)
                s_tp.pinc()
                W("act", s_tp, T0 + g + 1)

                def act_at(eng, g=g, bank=bank, aggT=aggT):
                    s_cp.inc(eng.activation(out=aggT[:, g * 512:(g + 1) * 512],
                                            in_=bank[0:32, :], func=ACOPY))
                op("act", act_at)
                s_cp.pinc()

        # dense tail
        W("pe", s_cp, s_cp.n)
        W("pe", s_x1, s_x1.n)
        X0 = s_x1.n
        for ch in range(NS):
            bank = pb[3 + (ch % 2)]
            if ch >= 2:
                W("pe", s_x1, X0 + ch - 1)

            def pe_tail(eng, ch=ch, bank=bank):
                sl = slice(ch * 512, (ch + 1) * 512)
                nc.tensor.matmul(bank[0:32, :], sb_wsm[:, 1, :], xT[0:33, sl],
                                 start=True, stop=False)
                nc.tensor.matmul(bank[0:32, :], sb_wsm[0:32, 2, :], aggT0[:, sl],
                                 start=False, stop=False)
                last = nc.tensor.matmul(bank[0:32, :], sb_wsm[0:32, 3, :], aggT1[:, sl],
                                        start=False, stop=True)
                s_mm.inc(last)
            op("pe", pe_tail)
            s_mm.pinc()
            W("act", s_mm, s_mm.n)

            def act_tail(eng, ch=ch, bank=bank):
                s_x1.inc(eng.activation(out=xT[0:32, ch * 512:(ch + 1) * 512],
                                        in_=bank[0:32, :], func=ACOPY))
            op("act", act_tail)
            s_x1.pinc()

        if layer == 1:
            S0 = s_sh.n
            X1 = X0
            for ch in range(NS):
                W("pe", s_x1, X1 + ch + 1)
                if ch >= 1:
                    W("pe", s_sh, S0 + ch)
                if ch == NS - 1 and cfg.shard_real < SP:
                    W("dve", s_x1, X1 + NS)
                    op("dve", lambda eng: s_dve.inc(eng.memset(xT[0:32, cfg.shard_real:SP], 0)))
                    s_dve.pinc()
                    W("pe", s_dve, s_dve.n)

                def pe_sh(eng, ch=ch):
                    last = None
                    for t in range(4):
                        last = nc.tensor.transpose(
                            out=pbx[:, t * 32:(t + 1) * 32],
                            in_=xT[0:32, ch * 512 + t * 128:ch * 512 + (t + 1) * 128],
                            identity=sb_id32[:])
                    s_tp.inc(last)
                op("pe", pe_sh)
                s_tp.pinc()
                W("act", s_tp, s_tp.n)
                if ch == 0:
                    W("act", s_gp, gp_shard_done)   # shard DMA of this layer done

                def act_sh2(eng, ch=ch):
                    s_sh.inc(eng.activation(
                        out=sb_shard[:, 4 * ch:4 * ch + 4, :].rearrange("p a b -> p (a b)"),
                        in_=pbx[:, 0:128], func=ACOPY))
                op("act", act_sh2)
                s_sh.pinc()

    emit_layer(1)
    emit_layer(2)

    # =======================================================
    # head
    # =======================================================
    W("pe", s_x1, s_x1.n)
    XH = s_x1.n
    GH = s_gp.n
    for ch in range(NS):
        bank = pb[3 + (ch % 2)]
        if ch >= 1:
            W("pe", s_x1, XH + 2 * ch)

        def pe_h1(eng, ch=ch, bank=bank):
            s_mm.inc(nc.tensor.matmul(bank[0:32, :], sb_wsm[:, 4, :],
                                      xT[0:33, ch * 512:(ch + 1) * 512], start=True, stop=True))
        op("pe", pe_h1)
        s_mm.pinc()
        W("act", s_mm, s_mm.n)
        op("act", lambda eng, bank=bank: s_lr.inc(eng.activation(
            out=sb_x3T[0:32, :], in_=bank[0:32, :], func=ACOPY)))
        s_lr.pinc()
        W("dve", s_lr, s_lr.n)

        def act_h1(eng, ch=ch, bank=bank):
            s_x1.inc(nc.vector.scalar_tensor_tensor(
                out=sb_x3T[0:32, :], in0=sb_x3T[0:32, :], scalar=0.01,
                in1=sb_x3T[0:32, :], op0=mybir.AluOpType.mult, op1=mybir.AluOpType.max))
        op("dve", act_h1)
        s_x1.pinc()
        W("pe", s_x1, s_x1.n)

        def pe_h2(eng, ch=ch, bank=bank):
            s_mm.inc(nc.tensor.matmul(bank[0:2, :], sb_wsm[:, 5, 0:2],
                                      sb_x3T[0:33, :], start=True, stop=True))
        op("pe", pe_h2)
        s_mm.pinc()
        W("act", s_mm, s_mm.n)
        if ch >= 2:
            W("act", s_gp, GH + (ch - 1) * 16)

        def act_h2(eng, ch=ch, bank=bank):
            s_x1.inc(eng.activation(out=sb_lg[:, ch % 2, :], in_=bank[0:2, :],
                                    func=ACOPY))
        op("act", act_h2)
        s_x1.pinc()
        W("gp", s_x1, s_x1.n)

        def gp_out(eng, ch=ch):
            s_gp.inc(eng.dma_start(out=out_ext[:, ch * 512:(ch + 1) * 512],
                                   in_=sb_lg[:, ch % 2, :]), 16)
        op("gp", gp_out)
        s_gp.pinc(16)
    W("gp", s_gp, s_gp.n)

    with nc.Block() as block:
        @block.sync
        def _(eng):
            for f in plan["sync"]:
                f(eng)

        @block.tensor
        def _(eng):
            for f in plan["pe"]:
                f(eng)

        @block.scalar
        def _(eng):
            for f in plan["act"]:
                f(eng)

        @block.vector
        def _(eng):
            for f in plan["dve"]:
                f(eng)

        @block.gpsimd
        def _(eng):
            for f in plan["gp"]:
                f(eng)

    nc.compile()
    nc._live_refs = (live, mmctx)
    return nc


# =======================================================
# Host side
# =======================================================
def _build_structures(edge_index, edge_type, shard_real=12500, shard_pad=12800):
    SP = shard_pad
    src = edge_index[0].astype(np.int64)
    dst = edge_index[1].astype(np.int64)
    et = edge_type.astype(np.int64)
    owner = dst // shard_real
    ldst = dst % shard_real
    trow = (src // shard_real) * SP + (src % shard_real)

    per_core = []
    for c in range(N_CORES):
        rels = []
        for r in range(NUM_REL):
            sel = (owner == c) & (et == r)
            l = ldst[sel]
            t = trow[sel]
            dcnt = np.bincount(l, minlength=SP)
            perm = np.argsort(-dcnt, kind="stable")
            rank = np.empty(SP, dtype=np.int64)
            rank[perm] = np.arange(SP)
            order = np.argsort(rank[l], kind="stable")
            l_s, t_s = l[order], t[order]
            s_sorted = rank[l_s]
            if len(l_s):
                newgrp = np.r_[True, s_sorted[1:] != s_sorted[:-1]]
                gidx = np.cumsum(newgrp) - 1
                starts = np.flatnonzero(newgrp)
                kpos = np.arange(len(l_s)) - starts[gidx]
            else:
                kpos = np.zeros(0, dtype=np.int64)
            maxd = int(dcnt.max()) if len(l) else 0
            Lk = np.array([(dcnt > k).sum() for k in range(maxd)], dtype=np.int64)
            rels.append(dict(dcnt=dcnt, perm=perm, rank=rank, s=s_sorted, k=kpos,
                             t=t_s, maxd=maxd, Lk=Lk))
        per_core.append(rels)

    maxd_g = [max(per_core[c][r]["maxd"] for c in range(N_CORES)) for r in range(NUM_REL)]
    c_r = []
    for r in range(NUM_REL):
        cks = []
        for k in range(maxd_g[r]):
            m = 1
            for c in range(N_CORES):
                Lk = per_core[c][r]["Lk"]
                if k < len(Lk):
                    m = max(m, int(np.ceil(Lk[k] / 128)))
            cks.append(m)
        c_r.append(cks)

    # piece decomposition (shared across cores)
    pieces = []
    colbase_r = []
    gcol = 0
    for r in range(NUM_REL):
        colbase = []
        plist = []
        cur_c0 = gcol
        cur_ck = 0
        cur_adds = []
        for k, ck in enumerate(c_r[r]):
            colbase.append(gcol)
            off = 0
            while off < ck:
                room = CBUF - cur_ck
                if room == 0:
                    plist.append((cur_c0, cur_ck, cur_adds))
                    cur_c0, cur_ck, cur_adds = cur_c0 + CBUF, 0, []
                    room = CBUF
                take = min(room, ck - off)
                cur_adds.append((off, cur_ck, take))
                cur_ck += take
                off += take
            gcol += ck
        if cur_ck:
            plist.append((cur_c0, cur_ck, cur_adds))
        pieces.append(plist)
        colbase_r.append(np.array(colbase, dtype=np.int64))

    cfg = Cfg(shard_real, SP, pieces, gcol)
    return cfg, per_core, colbase_r


def _prep(inputs, shard_real=12500, shard_pad=12800):
    SP = shard_pad
    cfg, per_core, colbase_r = _build_structures(
        inputs["edge_index"], inputs["edge_type"], shard_real, shard_pad)
    NT = cfg.nt
    CUNP = cfg.c_unp
    NG = cfg.n_gcols

    f32 = np.float32
    bf16 = ml_dtypes.bfloat16
    wd = np.zeros((768, 32), dtype=bf16); wd[:, 0:8] = inputs["Wd"].astype(bf16)
    wt = np.zeros((768, 32), dtype=bf16); wt[:, 8:16] = inputs["Wt"].astype(bf16)
    wnc = np.zeros((16, 32), dtype=bf16)
    wnc[0:6, 16:24] = inputs["Wn"].astype(bf16)
    wnc[6:9, 24:32] = inputs["Wc"].astype(bf16)
    bx = np.zeros(32, dtype=np.float32)
    bx[0:8] = inputs["bd"]; bx[8:16] = inputs["bt"]
    bx[16:24] = inputs["bn"]; bx[24:32] = inputs["bc"]
    wnc[9, :] = bx.astype(bf16)
    wsm = np.zeros((6, 33, 32), dtype=bf16)
    wsm[0, 0:32] = inputs["Wi"].astype(bf16)
    wsm[0, 32] = inputs["bi"].astype(bf16)
    wsm[1, 0:32] = inputs["Wroot"].astype(bf16)
    wsm[1, 32] = inputs["brgcn"].astype(bf16)
    wsm[2, 0:32] = inputs["Wrel"][0].astype(bf16)
    wsm[3, 0:32] = inputs["Wrel"][1].astype(bf16)
    wsm[4, 0:32] = inputs["Wo1"].astype(bf16)
    wsm[4, 32] = inputs["bo1"].astype(bf16)
    wsm[5, 0:32, 0:2] = inputs["Wo2"].astype(bf16)
    wsm[5, 32, 0:2] = inputs["bo2"].astype(bf16)
    id128 = np.eye(128, dtype=f32)
    id32 = np.eye(32, dtype=bf16)

    in_maps = []
    for c in range(N_CORES):
        r0, r1 = c * shard_real, (c + 1) * shard_real
        des = np.zeros((SP, 768), dtype=f32); des[0:shard_real] = inputs["des"][r0:r1]
        tw = np.zeros((SP, 768), dtype=f32); tw[0:shard_real] = inputs["tweet"][r0:r1]
        ncat = np.zeros((SP, 16), dtype=f32)
        ncat[0:shard_real, 0:6] = inputs["num_prop"][r0:r1]
        ncat[0:shard_real, 6:9] = inputs["cat_prop"][r0:r1]
        ncat[0:shard_real, 9] = 1.0

        # per-slot gather group indices (16-wrap layout) + 4 bucket masks
        gidx16 = np.zeros((16, 8 * NG), dtype=np.int16)
        cmask = np.zeros((128, 4, NG), dtype=bf16)
        for r in range(NUM_REL):
            d = per_core[c][r]
            if len(d["s"]) == 0:
                continue
            s = d["s"]; t = d["t"]; k = d["k"]
            cols = colbase_r[r][k] + s // 128
            p = s % 128
            cv = (1.0 / np.maximum(d["dcnt"][d["perm"]], 1.0)).astype(f32)
            gidx16[p % 16, 8 * cols + p // 16] = (t // 4).astype(np.int16)
            cmask[p, t % 4, cols] = cv[s].astype(bf16)
        # pack per piece: [8ck i16 idx | 4ck bf16 cmask]
        gconst = np.zeros((128, 12 * NG), dtype=np.int16)
        for r in range(NUM_REL):
            for (c0, ck, _) in cfg.pieces[r]:
                blk = np.tile(gidx16[:, 8 * c0:8 * (c0 + ck)], (8, 1))
                gconst[:, 12 * c0:12 * c0 + 8 * ck] = blk
                cm = np.ascontiguousarray(
                    cmask[:, :, c0:c0 + ck].reshape(128, 4 * ck)).view(np.int16)
                gconst[:, 12 * c0 + 8 * ck:12 * (c0 + ck)] = cm

        # unpermute gather indices: canonical slot n reads scratch[rank[n]]
        uidx = np.zeros((16, NUM_REL, 8 * CUNP), dtype=np.int16)
        n = np.arange(SP)
        for r in range(NUM_REL):
            d = per_core[c][r]
            uidx[n % 16, r, n // 16] = d["rank"][n].astype(np.int16)
        uidx = np.tile(uidx, (8, 1, 1))

        in_maps.append({
            "des": des, "tweet": tw, "numcat": ncat,
            "gconst": gconst, "uidx": uidx,
            "wd": wd, "wt": wt, "wnc": wnc, "wsm": wsm,
            "id128": id128, "id32": id32,
        })
    return cfg, in_maps


_CACHE = {}


def kernel(**inputs):
    cfg, in_maps = _prep(inputs)
    key = tuple((c0, ck) for r in range(NUM_REL) for (c0, ck, _) in cfg.pieces[r])
    if key not in _CACHE:
        _CACHE[key] = build_bass(cfg)
    nc = _CACHE[key]
    res = run_bass_kernel_spmd(nc, in_maps, list(range(N_CORES)))
    outs = []
    for c in range(N_CORES):
        o = res.results[c]["out"]
        outs.append(o.T[0:cfg.shard_real])
    return np.ascontiguousarray(np.concatenate(outs, axis=0).astype(np.float32))


# revision 17
# speedup vs baseline: 1.4148x; 1.2415x over previous
"""BotRGCN on 8 trn2 NeuronCores (SPMD, raw Bacc).

Nodes row-sharded 8 ways (12500/core, padded to 12800). Phase A
(768->8 projections etc.) fully sharded with PE transposes + bf16
matmuls. RGCN layers: AllGather bf16 node features -> shared gather
table viewed as 256B groups of 4 node-rows; per-relation degree-sorted
ELL slot gathers via big SWDGE dma_gather instructions (int16 group
indices); DVE masked select-add (host-precomputed bf16 masks fold the
1/cnt mean scaling); unpermute via one dma_gather from a 256B-row
scratch; dense matmuls in feature-major (^T) space.
"""
import os
import sys
sys.path.insert(0, "/opt/trn_rl_repo")
from contextlib import ExitStack

# ablation flags (timing experiments): comma-separated list in BASS_ABL
ABL = set(os.environ.get("BASS_ABL", "").split(","))

import numpy as np
import ml_dtypes

from concourse import bacc, bass, mybir
from concourse import library_config
from concourse.bass_utils import run_bass_kernel_spmd

F32 = mybir.dt.float32
BF16 = mybir.dt.bfloat16
I16 = mybir.dt.int16
LRELU = mybir.ActivationFunctionType.Lrelu
ACOPY = mybir.ActivationFunctionType.Copy

N_CORES = 8
NUM_REL = 2
CBUF = 96          # gather piece size (ELL columns per dma_gather)


class Cfg:
    def __init__(self, shard_real, shard_pad, pieces, n_gcols):
        self.shard_real = shard_real
        self.shard_pad = shard_pad
        self.n_super = shard_pad // 512
        self.nt = shard_pad // 128
        self.c_unp = shard_pad // 128
        self.tabv = N_CORES * shard_pad
        # pieces[r] = list of (col0, ck, adds) ; adds = [(agg_blk0, msg_col0, nblk)]
        self.pieces = pieces
        self.n_gcols = n_gcols


NQ = 4   # SWDGE queues for gather parallelism


def build_bass(cfg: Cfg):
    nc = bacc.Bacc("TRN2", debug=False, num_swdge_queues=NQ)
    mmctx = ExitStack()
    SP = cfg.shard_pad
    NT = cfg.nt
    NS = cfg.n_super
    TABV = cfg.tabv
    CUNP = cfg.c_unp
    NG = cfg.n_gcols

    des_in = nc.declare_dram_parameter("des", [SP, 768], F32, isOutput=False)
    tw_in = nc.declare_dram_parameter("tweet", [SP, 768], F32, isOutput=False)
    ncat_in = nc.declare_dram_parameter("numcat", [SP, 16], F32, isOutput=False)
    gconst_in = nc.declare_dram_parameter("gconst", [128, 12 * NG], I16, isOutput=False)
    uidx_in = nc.declare_dram_parameter("uidx", [128, NUM_REL, 8 * CUNP], I16, isOutput=False)
    wd_in = nc.declare_dram_parameter("wd", [768, 32], BF16, isOutput=False)
    wt_in = nc.declare_dram_parameter("wt", [768, 32], BF16, isOutput=False)
    wnc_in = nc.declare_dram_parameter("wnc", [16, 32], BF16, isOutput=False)
    wsm_in = nc.declare_dram_parameter("wsm", [6, 33, 32], BF16, isOutput=False)
    id128_in = nc.declare_dram_parameter("id128", [128, 128], F32, isOutput=False)
    id32_in = nc.declare_dram_parameter("id32", [32, 32], BF16, isOutput=False)
    out_ext = nc.declare_dram_parameter("out", [2, SP], F32, isOutput=True)

    shard_ag = nc.dram_tensor("shard_ag", [SP, 32], BF16)
    table = nc.dram_tensor("table", [TABV, 32], BF16, addr_space="Shared")
    scratch = [nc.dram_tensor(f"scratch{r}", [SP, 64], F32) for r in range(NUM_REL)]

    live = []

    def sb(name, shape, dt):
        cm = nc.sbuf_tensor(name, shape, dt)
        t = cm.__enter__()
        live.append(cm)
        return t

    def psum(name, shape):
        cm = nc.psum_tensor(name, shape, F32)
        t = cm.__enter__()
        live.append(cm)
        return t

    sb_des = sb("sb_des", [128, 2, 4, 768], F32)
    sb_tw = sb("sb_tw", [128, 2, 4, 768], F32)
    sb_nc = sb("sb_nc", [128, 2, 4, 16], F32)
    sb_desT = sb("sb_desT", [128, 6, 512], BF16)
    sb_twT = sb("sb_twT", [128, 6, 512], BF16)
    sb_ncT = sb("sb_ncT", [16, 512], BF16)
    xT = sb("xT", [33, SP], BF16)
    aggT0 = sb("aggT0", [32, SP], BF16)
    aggT1 = sb("aggT1", [32, SP], BF16)
    agg = sb("agg", [128, NT * 32], F32)
    # gather piece buffers alias the phase-A des/tweet staging (consumed by then)
    msgs2 = [sb_des[:, :, :, :].bitcast(BF16).rearrange("p a b c -> p (a b c)")
             .rearrange("p (a b) -> p a b", b=128),
             sb_tw[:, :, :, :].bitcast(BF16).rearrange("p a b c -> p (a b c)")
             .rearrange("p (a b) -> p a b", b=128)]
    tmp2 = [sb(f"tmp{i}", [128, CBUF, 32], BF16) for i in range(2)]
    sb_gc = sb("sb_gc", [128, 2, 12 * CBUF], I16)
    sb_uidx = sb("sb_uidx", [128, NUM_REL, 8 * CUNP], I16)
    unp = sb("unp", [128, CUNP, 64], F32)
    sb_shard = sb("sb_shard", [128, NT, 32], BF16)
    sb_wd = sb("sb_wd", [128, 6, 32], BF16)
    sb_wt = sb("sb_wt", [128, 6, 32], BF16)
    sb_wnc = sb("sb_wnc", [16, 32], BF16)
    sb_wsm = sb("sb_wsm", [33, 6, 32], BF16)
    sb_id128 = sb("sb_id128", [128, 128], F32)
    sb_id32 = sb("sb_id32", [32, 32], BF16)
    sb_x3T = sb("sb_x3T", [33, 512], BF16)
    sb_lg = sb("sb_lg", [2, 2, 512], F32)

    pb = [psum(f"pb{i}", [128, 512]) for i in range(8)]
    pbx = pb[5][:, :].bitcast(BF16)

    table4 = table.ap().rearrange("(g f) d -> g (f d)", f=4)   # [TABV/4, 128]

    plan = {"sync": [], "pe": [], "act": [], "dve": [], "gp": []}

    def op(engine, fn):
        plan[engine].append(fn)

    class Sem:
        def __init__(self, name):
            cm = nc.semaphore(name)
            self.h = cm.__enter__()
            live.append(cm)
            self.n = 0

        def inc(self, inst, k=1):
            inst.then_inc(self.h, k)

        def pinc(self, k=1):
            self.n += k
            return self.n

    s_load = Sem("s_load")
    s_ld = [Sem("s_ld0"), Sem("s_ld1")]
    s_lr = Sem("s_lr")
    s_gq = [Sem(f"s_gq{q}") for q in range(NQ)]   # per-SWDGE-queue gather sems
    s_gc = Sem("s_gc")      # gconst piece loads
    s_tp = Sem("s_tp")
    s_cp = Sem("s_cp")
    s_mm = Sem("s_mm")
    s_x1 = Sem("s_x1")
    s_gp = Sem("s_gp")      # gp-engine bulk DMAs (SWDGE)
    s_cc = Sem("s_cc")
    s_dve = Sem("s_dve")
    s_sh = Sem("s_sh")

    def W(engine, sem, val):
        if val > 0:
            op(engine, lambda eng, s=sem, v=val: eng.wait_ge(s.h, v))

    # ---------------- constants ----------------
    def c_loads(eng):
        eng.dma_start(out=sb_uidx[:], in_=uidx_in[:, :, :]).then_inc(s_load.h, 16)
        eng.dma_start(out=sb_wd[:], in_=wd_in.ap().rearrange("(c p) m -> p c m", p=128)).then_inc(s_load.h, 16)
        eng.dma_start(out=sb_wt[:], in_=wt_in.ap().rearrange("(c p) m -> p c m", p=128)).then_inc(s_load.h, 16)
        eng.dma_start(out=sb_wnc[:], in_=wnc_in[:, :]).then_inc(s_load.h, 16)
        eng.dma_start(out=sb_wsm[:], in_=wsm_in.ap().rearrange("c p m -> p c m")).then_inc(s_load.h, 16)
        eng.dma_start(out=sb_id128[:], in_=id128_in[:, :]).then_inc(s_load.h, 16)
        eng.dma_start(out=sb_id32[:], in_=id32_in[:, :]).then_inc(s_load.h, 16)
    op("sync", c_loads)
    s_load.n += 7 * 16
    NCONST = s_load.n

    op("gp", lambda eng: eng.load_library(library_config.mlp))

    def init_ones(eng):
        eng.memset(xT[32:33, :], 1.0)
        s_dve.inc(eng.memset(sb_x3T[32:33, :], 1.0))
    op("dve", init_ones)
    s_dve.pinc()
    NINIT = s_dve.n

    # =======================================================
    # Phase A
    # =======================================================
    for i in range(NS):
        buf = i % 2
        if i >= 2:
            W("sync", s_mm, 2 * (i - 1))

        def ld(eng, i=i, buf=buf):
            r0 = i * 512
            eng.dma_start(out=sb_des[:, buf, :, :],
                          in_=des_in[r0:r0 + 512, :].rearrange("(t p) c -> p t c", p=128)
                          ).then_inc(s_ld[buf].h, 16)
            eng.dma_start(out=sb_tw[:, buf, :, :],
                          in_=tw_in[r0:r0 + 512, :].rearrange("(t p) c -> p t c", p=128)
                          ).then_inc(s_ld[buf].h, 16)
            eng.dma_start(out=sb_nc[:, buf, :, :],
                          in_=ncat_in[r0:r0 + 512, :].rearrange("(t p) c -> p t c", p=128)
                          ).then_inc(s_ld[buf].h, 16)
        op("sync", ld)
        s_ld[buf].n += 48

        # ---- PE ----
        if i == 0:
            W("pe", s_load, NCONST)
        W("pe", s_ld[buf], 48 * (i // 2 + 1))
        if i >= 1:
            W("pe", s_cp, i * 5)          # ACT copy-rounds of i-1 done (psum WAR)
            W("pe", s_x1, 2 * i)          # act_x1(i-1) consumed pb7

        def pe_nc(eng, buf=buf):
            last = None
            for t in range(4):
                last = nc.tensor.transpose(out=pb[7][0:16, t * 128:(t + 1) * 128],
                                           in_=sb_nc[:, buf, t, :], identity=sb_id128[:])
            s_tp.inc(last)
        op("pe", pe_nc)
        s_tp.pinc()

        for (src, bank0, waitv) in ((sb_des, 0, i * 5 + 2), (sb_tw, 3, i * 5 + 4)):
            if bank0 == 3 and i >= 1:
                W("pe", s_sh, i)          # act_sh(i-1) consumed pb5-alias
            for rnd in range(2):
                if rnd == 1:
                    W("pe", s_cp, waitv)  # ACT copied round 0 of this tensor

                def pe_tp(eng, src=src, bank0=bank0, rnd=rnd, buf=buf):
                    last = None
                    for cc in range(3):
                        c = rnd * 3 + cc
                        for t in range(4):
                            last = nc.tensor.transpose(
                                out=pb[bank0 + cc][:, t * 128:(t + 1) * 128],
                                in_=src[:, buf, t, c * 128:(c + 1) * 128],
                                identity=sb_id128[:])
                    s_tp.inc(last)
                op("pe", pe_tp)
                s_tp.pinc()

        W("pe", s_cp, i * 5 + 5)

        def pe_mm(eng):
            for c in range(6):
                nc.tensor.matmul(pb[6][0:32, :], sb_wd[:, c, :], sb_desT[:, c, :],
                                 start=(c == 0), stop=False)
            for c in range(6):
                nc.tensor.matmul(pb[6][0:32, :], sb_wt[:, c, :], sb_twT[:, c, :],
                                 start=False, stop=False)
            last = nc.tensor.matmul(pb[6][0:32, :], sb_wnc[:, :], sb_ncT[:, :],
                                    start=False, stop=True)
            s_mm.inc(last)
        op("pe", pe_mm)
        s_mm.pinc()

        W("pe", s_x1, 2 * i + 1)
        if i == 0:
            W("pe", s_dve, NINIT)

        def pe_wi(eng, i=i):
            last = nc.tensor.matmul(pb[7][0:32, :], sb_wsm[:, 0, :],
                                    xT[0:33, i * 512:(i + 1) * 512], start=True, stop=True)
            s_mm.inc(last)
        op("pe", pe_wi)
        s_mm.pinc()

        W("pe", s_x1, 2 * i + 2)
        if i >= 1:
            W("pe", s_sh, i)              # act_sh(i-1) consumed pb0
        if i == NS - 1 and cfg.shard_real < SP:
            W("pe", s_dve, NINIT + 1)

        def pe_x1t(eng, i=i):
            last = None
            for t in range(4):
                last = nc.tensor.transpose(
                    out=pbx[:, t * 32:(t + 1) * 32],
                    in_=xT[0:32, i * 512 + t * 128:i * 512 + (t + 1) * 128],
                    identity=sb_id32[:])
            s_tp.inc(last)
        op("pe", pe_x1t)
        s_tp.pinc()

        # ---- ACT ----
        W("act", s_tp, i * 6 + 1)
        op("act", lambda eng: s_cp.inc(eng.activation(out=sb_ncT[:, :], in_=pb[7][0:16, :],
                                                      func=ACOPY)))
        s_cp.pinc()
        k = 0
        for (dstT, bank0) in ((sb_desT, 0), (sb_twT, 3)):
            for rnd in range(2):
                k += 1
                W("act", s_tp, i * 6 + 1 + k)

                def act_cp(eng, dstT=dstT, bank0=bank0, rnd=rnd):
                    last = None
                    for cc in range(3):
                        c = rnd * 3 + cc
                        last = eng.activation(out=dstT[:, c, :], in_=pb[bank0 + cc][:, :],
                                              func=ACOPY)
                    s_cp.inc(last)
                op("act", act_cp)
                s_cp.pinc()

        W("act", s_mm, 2 * i + 1)
        op("act", lambda eng, i=i: s_lr.inc(eng.activation(
            out=xT[0:32, i * 512:(i + 1) * 512], in_=pb[6][0:32, :], func=ACOPY)))
        s_lr.pinc()
        W("dve", s_lr, s_lr.n)

        def act_x(eng, i=i):
            sl = xT[0:32, i * 512:(i + 1) * 512]
            s_x1.inc(nc.vector.scalar_tensor_tensor(
                out=sl, in0=sl, scalar=0.01, in1=sl,
                op0=mybir.AluOpType.mult, op1=mybir.AluOpType.max))
        op("dve", act_x)
        s_x1.pinc()
        W("act", s_mm, 2 * i + 2)
        op("act", lambda eng, i=i: s_lr.inc(eng.activation(
            out=xT[0:32, i * 512:(i + 1) * 512], in_=pb[7][0:32, :], func=ACOPY)))
        s_lr.pinc()
        W("dve", s_lr, s_lr.n)

        def act_x1(eng, i=i):
            sl = xT[0:32, i * 512:(i + 1) * 512]
            s_x1.inc(nc.vector.scalar_tensor_tensor(
                out=sl, in0=sl, scalar=0.01, in1=sl,
                op0=mybir.AluOpType.mult, op1=mybir.AluOpType.max))
        op("dve", act_x1)
        s_x1.pinc()
        if i == NS - 1 and cfg.shard_real < SP:
            W("dve", s_x1, 2 * NS)
            op("dve", lambda eng: s_dve.inc(eng.memset(xT[0:32, cfg.shard_real:SP], 0)))
            s_dve.pinc()
        W("act", s_tp, i * 6 + 6)

        def act_sh(eng, i=i):
            s_sh.inc(eng.activation(
                out=sb_shard[:, 4 * i:4 * i + 4, :].rearrange("p a b -> p (a b)"),
                in_=pbx[:, 0:128], func=ACOPY))
        op("act", act_sh)
        s_sh.pinc()

    # =======================================================
    # RGCN layers
    # =======================================================
    glob = {"gpi": 0}
    piece_dve_after = []   # s_dve count after global piece j's select-adds

    def emit_layer(layer):
        W("gp", s_sh, s_sh.n)
        op("gp", lambda eng: s_gp.inc(
            eng.dma_start(out=shard_ag[:, :].rearrange("(t p) d -> p t d", p=128),
                          in_=sb_shard[:, :, :]), 16))
        s_gp.pinc(16)
        gp_shard_done = s_gp.n
        W("gp", s_gp, gp_shard_done)
        if "noag" not in ABL:
            op("gp", lambda eng: s_cc.inc(eng.collective_compute(
                "AllGather", mybir.AluOpType.bypass,
                ins=[shard_ag[:, :]], outs=[table[:, :]],
                replica_groups=[list(range(N_CORES))])))
            s_cc.pinc()
        CCN = s_cc.n

        for r in range(NUM_REL):
            # memset agg; waits prior relation's scratch write (agg WAR)
            W("dve", s_gp, s_gp.n)
            op("dve", lambda eng: s_dve.inc(eng.memset(agg[:, :], 0)))
            s_dve.pinc()

            for pi, (c0, ck, adds) in enumerate(cfg.pieces[r]):
                if "nogather" in ABL:
                    break
                j = glob["gpi"]
                glob["gpi"] += 1
                pbuf = j % 2
                # stream this piece's idx+cmask block
                if j >= 2:
                    W("sync", s_dve, piece_dve_after[j - 2])
                op("sync", lambda eng, c0=c0, ck=ck, pbuf=pbuf: s_gc.inc(
                    eng.dma_start(out=sb_gc[:, pbuf, 0:12 * ck],
                                  in_=gconst_in[:, 12 * c0:12 * (c0 + ck)]), 16))
                s_gc.pinc(16)
                gcv = s_gc.n
                # gather on gp
                W("gp", s_gc, gcv)
                if pi == 0:
                    W("gp", s_cc, CCN)
                if j >= 2:
                    W("gp", s_dve, piece_dve_after[j - 2])   # msgs/gc WAR

                q = j % NQ

                def gth(eng, ck=ck, pbuf=pbuf, q=q):
                    s_gq[q].inc(eng.dma_gather(
                        out_ap=msgs2[pbuf][:, 0:ck, :],
                        in_ap=table4,
                        idxs_ap=sb_gc[:, pbuf, 0:8 * ck],
                        num_idxs=128 * ck,
                        num_idxs_reg=128 * ck,
                        elem_size=128,
                        single_packet=False,
                        queue_num=q,
                    ), 16)
                op("gp", gth)
                s_gq[q].pinc(16)
                GQ = s_gq[q].n
                # DVE select-adds
                W("dve", s_gq[q], GQ)

                def sel(eng, ck=ck, adds=adds, pbuf=pbuf):
                    cmbase = 8 * ck
                    gcb = sb_gc[:, pbuf, :].bitcast(BF16)   # [128, 12*CBUF]
                    last = None
                    for b in range(4):
                        tb = tmp2[b % 2]
                        if b > 0:
                            eng.drain()
                        nc.vector.tensor_tensor(
                            out=tb[:, 0:ck, :],
                            in0=msgs2[pbuf][:, 0:ck, 32 * b:32 * b + 32],
                            in1=gcb[:, cmbase + b * ck:cmbase + (b + 1) * ck]
                                .to_broadcast([128, ck, 32]),
                            op=mybir.AluOpType.mult)
                        for (ab, mb, nb) in adds:
                            eng.drain()
                            last = nc.vector.tensor_tensor(
                                out=agg[:, ab * 32:(ab + nb) * 32],
                                in0=agg[:, ab * 32:(ab + nb) * 32],
                                in1=tb[:, mb:mb + nb, :].rearrange("p a b -> p (a b)"),
                                op=mybir.AluOpType.add)
                    s_dve.inc(last)
                op("dve", sel)
                s_dve.pinc()
                piece_dve_after.append(s_dve.n)

            # relation done: agg (sorted order) -> scratch (256B rows)
            W("gp", s_dve, s_dve.n)
            op("gp", lambda eng, r=r: s_gp.inc(
                eng.dma_start(out=scratch[r][:, 0:32].rearrange("(t p) d -> p t d", p=128),
                              in_=agg[:, :].rearrange("p (t d) -> p t d", d=32)), 16))
            s_gp.pinc(16)
            W("gp", s_gp, s_gp.n)
            W("gp", s_tp, s_tp.n)   # prior relation's unp transposes done (unp WAR)

            uq = glob["gpi"] % NQ
            glob["gpi"] += 1
            if "nounp" not in ABL:
                def unp_g(eng, r=r, uq=uq):
                    s_gq[uq].inc(eng.dma_gather(
                        out_ap=unp[:, :, :],
                        in_ap=scratch[r].ap(),
                        idxs_ap=sb_uidx[:, r, :],
                        num_idxs=SP,
                        num_idxs_reg=SP,
                        elem_size=64,
                        single_packet=False,
                        queue_num=uq,
                    ), 16)
                op("gp", unp_g)
                s_gq[uq].pinc(16)
            UNPQ = s_gq[uq].n
            piece_dve_after.append(s_dve.n)

            # transpose unp (canonical node-major fp32) -> aggT (bf16 ^T)
            aggT = aggT0 if r == 0 else aggT1
            W("pe", s_gq[uq], UNPQ)
            C0 = s_cp.n
            T0 = s_tp.n
            for g in range(NT // 4):
                bank = pb[1 + (g % 2)]
                W("pe", s_cp, C0 + g - 1 if g >= 2 else C0)

                def pe_at(eng, g=g, bank=bank):
                    last = None
                    for t in range(4):
                        n = g * 4 + t
                        last = nc.tensor.transpose(out=bank[0:32, t * 128:(t + 1) * 128],
                                                   in_=unp[:, n, 0:32],
                                                   identity=sb_id128[:])
                    s_tp.inc(last)
                op("pe", pe_at)
                s_tp.pinc()
                W("act", s_tp, T0 + g + 1)

                def act_at(eng, g=g, bank=bank, aggT=aggT):
                    s_cp.inc(eng.activation(out=aggT[:, g * 512:(g + 1) * 512],
                                            in_=bank[0:32, :], func=ACOPY))
                op("act", act_at)
                s_cp.pinc()

        # dense tail
        W("pe", s_cp, s_cp.n)
        W("pe", s_x1, s_x1.n)
        X0 = s_x1.n
        for ch in range(NS):
            bank = pb[3 + (ch % 2)]
            if ch >= 2:
                W("pe", s_x1, X0 + ch - 1)

            def pe_tail(eng, ch=ch, bank=bank):
                sl = slice(ch * 512, (ch + 1) * 512)
                nc.tensor.matmul(bank[0:32, :], sb_wsm[:, 1, :], xT[0:33, sl],
                                 start=True, stop=False)
                nc.tensor.matmul(bank[0:32, :], sb_wsm[0:32, 2, :], aggT0[:, sl],
                                 start=False, stop=False)
                last = nc.tensor.matmul(bank[0:32, :], sb_wsm[0:32, 3, :], aggT1[:, sl],
                                        start=False, stop=True)
                s_mm.inc(last)
            op("pe", pe_tail)
            s_mm.pinc()
            W("act", s_mm, s_mm.n)

            def act_tail(eng, ch=ch, bank=bank):
                s_x1.inc(eng.activation(out=xT[0:32, ch * 512:(ch + 1) * 512],
                                        in_=bank[0:32, :], func=ACOPY))
            op("act", act_tail)
            s_x1.pinc()

        if layer == 1:
            S0 = s_sh.n
            X1 = X0
            for ch in range(NS):
                W("pe", s_x1, X1 + ch + 1)
                if ch >= 1:
                    W("pe", s_sh, S0 + ch)
                if ch == NS - 1 and cfg.shard_real < SP:
                    W("dve", s_x1, X1 + NS)
                    op("dve", lambda eng: s_dve.inc(eng.memset(xT[0:32, cfg.shard_real:SP], 0)))
                    s_dve.pinc()
                    W("pe", s_dve, s_dve.n)

                def pe_sh(eng, ch=ch):
                    last = None
                    for t in range(4):
                        last = nc.tensor.transpose(
                            out=pbx[:, t * 32:(t + 1) * 32],
                            in_=xT[0:32, ch * 512 + t * 128:ch * 512 + (t + 1) * 128],
                            identity=sb_id32[:])
                    s_tp.inc(last)
                op("pe", pe_sh)
                s_tp.pinc()
                W("act", s_tp, s_tp.n)
                if ch == 0:
                    W("act", s_gp, gp_shard_done)   # shard DMA of this layer done

                def act_sh2(eng, ch=ch):
                    s_sh.inc(eng.activation(
                        out=sb_shard[:, 4 * ch:4 * ch + 4, :].rearrange("p a b -> p (a b)"),
                        in_=pbx[:, 0:128], func=ACOPY))
                op("act", act_sh2)
                s_sh.pinc()

    emit_layer(1)
    emit_layer(2)

    # =======================================================
    # head
    # =======================================================
    W("pe", s_x1, s_x1.n)
    XH = s_x1.n
    GH = s_gp.n
    for ch in range(NS):
        bank = pb[3 + (ch % 2)]
        if ch >= 1:
            W("pe", s_x1, XH + 2 * ch)

        def pe_h1(eng, ch=ch, bank=bank):
            s_mm.inc(nc.tensor.matmul(bank[0:32, :], sb_wsm[:, 4, :],
                                      xT[0:33, ch * 512:(ch + 1) * 512], start=True, stop=True))
        op("pe", pe_h1)
        s_mm.pinc()
        W("act", s_mm, s_mm.n)
        op("act", lambda eng, bank=bank: s_lr.inc(eng.activation(
            out=sb_x3T[0:32, :], in_=bank[0:32, :], func=ACOPY)))
        s_lr.pinc()
        W("dve", s_lr, s_lr.n)

        def act_h1(eng, ch=ch, bank=bank):
            s_x1.inc(nc.vector.scalar_tensor_tensor(
                out=sb_x3T[0:32, :], in0=sb_x3T[0:32, :], scalar=0.01,
                in1=sb_x3T[0:32, :], op0=mybir.AluOpType.mult, op1=mybir.AluOpType.max))
        op("dve", act_h1)
        s_x1.pinc()
        W("pe", s_x1, s_x1.n)

        def pe_h2(eng, ch=ch, bank=bank):
            s_mm.inc(nc.tensor.matmul(bank[0:2, :], sb_wsm[:, 5, 0:2],
                                      sb_x3T[0:33, :], start=True, stop=True))
        op("pe", pe_h2)
        s_mm.pinc()
        W("act", s_mm, s_mm.n)
        if ch >= 2:
            W("act", s_gp, GH + (ch - 1) * 16)

        def act_h2(eng, ch=ch, bank=bank):
            s_x1.inc(eng.activation(out=sb_lg[:, ch % 2, :], in_=bank[0:2, :],
                                    func=ACOPY))
        op("act", act_h2)
        s_x1.pinc()
        W("gp", s_x1, s_x1.n)

        def gp_out(eng, ch=ch):
            s_gp.inc(eng.dma_start(out=out_ext[:, ch * 512:(ch + 1) * 512],
                                   in_=sb_lg[:, ch % 2, :]), 16)
        op("gp", gp_out)
        s_gp.pinc(16)
    W("gp", s_gp, s_gp.n)

    with nc.Block() as block:
        @block.sync
        def _(eng):
            for f in plan["sync"]:
                f(eng)

        @block.tensor
        def _(eng):
            for f in plan["pe"]:
                f(eng)

        @block.scalar
        def _(eng):
            for f in plan["act"]:
                f(eng)

        @block.vector
        def _(eng):
            for f in plan["dve"]:
                f(eng)

        @block.gpsimd
        def _(eng):
            for f in plan["gp"]:
                f(eng)

    nc.compile()
    nc._live_refs = (live, mmctx)
    return nc


# =======================================================
# Host side
# =======================================================
def _build_structures(edge_index, edge_type, shard_real=12500, shard_pad=12800):
    SP = shard_pad
    src = edge_index[0].astype(np.int64)
    dst = edge_index[1].astype(np.int64)
    et = edge_type.astype(np.int64)
    owner = dst // shard_real
    ldst = dst % shard_real
    trow = (src // shard_real) * SP + (src % shard_real)

    per_core = []
    for c in range(N_CORES):
        rels = []
        for r in range(NUM_REL):
            sel = (owner == c) & (et == r)
            l = ldst[sel]
            t = trow[sel]
            dcnt = np.bincount(l, minlength=SP)
            perm = np.argsort(-dcnt, kind="stable")
            rank = np.empty(SP, dtype=np.int64)
            rank[perm] = np.arange(SP)
            order = np.argsort(rank[l], kind="stable")
            l_s, t_s = l[order], t[order]
            s_sorted = rank[l_s]
            if len(l_s):
                newgrp = np.r_[True, s_sorted[1:] != s_sorted[:-1]]
                gidx = np.cumsum(newgrp) - 1
                starts = np.flatnonzero(newgrp)
                kpos = np.arange(len(l_s)) - starts[gidx]
            else:
                kpos = np.zeros(0, dtype=np.int64)
            maxd = int(dcnt.max()) if len(l) else 0
            Lk = np.array([(dcnt > k).sum() for k in range(maxd)], dtype=np.int64)
            rels.append(dict(dcnt=dcnt, perm=perm, rank=rank, s=s_sorted, k=kpos,
                             t=t_s, maxd=maxd, Lk=Lk))
        per_core.append(rels)

    maxd_g = [max(per_core[c][r]["maxd"] for c in range(N_CORES)) for r in range(NUM_REL)]
    c_r = []
    for r in range(NUM_REL):
        cks = []
        for k in range(maxd_g[r]):
            m = 1
            for c in range(N_CORES):
                Lk = per_core[c][r]["Lk"]
                if k < len(Lk):
                    m = max(m, int(np.ceil(Lk[k] / 128)))
            cks.append(m)
        c_r.append(cks)

    # piece decomposition (shared across cores)
    pieces = []
    colbase_r = []
    gcol = 0
    for r in range(NUM_REL):
        colbase = []
        plist = []
        cur_c0 = gcol
        cur_ck = 0
        cur_adds = []
        for k, ck in enumerate(c_r[r]):
            colbase.append(gcol)
            off = 0
            while off < ck:
                room = CBUF - cur_ck
                if room == 0:
                    plist.append((cur_c0, cur_ck, cur_adds))
                    cur_c0, cur_ck, cur_adds = cur_c0 + CBUF, 0, []
                    room = CBUF
                take = min(room, ck - off)
                cur_adds.append((off, cur_ck, take))
                cur_ck += take
                off += take
            gcol += ck
        if cur_ck:
            plist.append((cur_c0, cur_ck, cur_adds))
        pieces.append(plist)
        colbase_r.append(np.array(colbase, dtype=np.int64))

    cfg = Cfg(shard_real, SP, pieces, gcol)
    return cfg, per_core, colbase_r


def _prep(inputs, shard_real=12500, shard_pad=12800):
    SP = shard_pad
    cfg, per_core, colbase_r = _build_structures(
        inputs["edge_index"], inputs["edge_type"], shard_real, shard_pad)
    NT = cfg.nt
    CUNP = cfg.c_unp
    NG = cfg.n_gcols

    f32 = np.float32
    bf16 = ml_dtypes.bfloat16
    wd = np.zeros((768, 32), dtype=bf16); wd[:, 0:8] = inputs["Wd"].astype(bf16)
    wt = np.zeros((768, 32), dtype=bf16); wt[:, 8:16] = inputs["Wt"].astype(bf16)
    wnc = np.zeros((16, 32), dtype=bf16)
    wnc[0:6, 16:24] = inputs["Wn"].astype(bf16)
    wnc[6:9, 24:32] = inputs["Wc"].astype(bf16)
    bx = np.zeros(32, dtype=np.float32)
    bx[0:8] = inputs["bd"]; bx[8:16] = inputs["bt"]
    bx[16:24] = inputs["bn"]; bx[24:32] = inputs["bc"]
    wnc[9, :] = bx.astype(bf16)
    wsm = np.zeros((6, 33, 32), dtype=bf16)
    wsm[0, 0:32] = inputs["Wi"].astype(bf16)
    wsm[0, 32] = inputs["bi"].astype(bf16)
    wsm[1, 0:32] = inputs["Wroot"].astype(bf16)
    wsm[1, 32] = inputs["brgcn"].astype(bf16)
    wsm[2, 0:32] = inputs["Wrel"][0].astype(bf16)
    wsm[3, 0:32] = inputs["Wrel"][1].astype(bf16)
    wsm[4, 0:32] = inputs["Wo1"].astype(bf16)
    wsm[4, 32] = inputs["bo1"].astype(bf16)
    wsm[5, 0:32, 0:2] = inputs["Wo2"].astype(bf16)
    wsm[5, 32, 0:2] = inputs["bo2"].astype(bf16)
    id128 = np.eye(128, dtype=f32)
    id32 = np.eye(32, dtype=bf16)

    in_maps = []
    for c in range(N_CORES):
        r0, r1 = c * shard_real, (c + 1) * shard_real
        des = np.zeros((SP, 768), dtype=f32); des[0:shard_real] = inputs["des"][r0:r1]
        tw = np.zeros((SP, 768), dtype=f32); tw[0:shard_real] = inputs["tweet"][r0:r1]
        ncat = np.zeros((SP, 16), dtype=f32)
        ncat[0:shard_real, 0:6] = inputs["num_prop"][r0:r1]
        ncat[0:shard_real, 6:9] = inputs["cat_prop"][r0:r1]
        ncat[0:shard_real, 9] = 1.0

        # per-slot gather group indices (16-wrap layout) + 4 bucket masks
        gidx16 = np.zeros((16, 8 * NG), dtype=np.int16)
        cmask = np.zeros((128, 4, NG), dtype=bf16)
        for r in range(NUM_REL):
            d = per_core[c][r]
            if len(d["s"]) == 0:
                continue
            s = d["s"]; t = d["t"]; k = d["k"]
            cols = colbase_r[r][k] + s // 128
            p = s % 128
            cv = (1.0 / np.maximum(d["dcnt"][d["perm"]], 1.0)).astype(f32)
            gidx16[p % 16, 8 * cols + p // 16] = (t // 4).astype(np.int16)
            cmask[p, t % 4, cols] = cv[s].astype(bf16)
        # pack per piece: [8ck i16 idx | 4ck bf16 cmask]
        gconst = np.zeros((128, 12 * NG), dtype=np.int16)
        for r in range(NUM_REL):
            for (c0, ck, _) in cfg.pieces[r]:
                blk = np.tile(gidx16[:, 8 * c0:8 * (c0 + ck)], (8, 1))
                gconst[:, 12 * c0:12 * c0 + 8 * ck] = blk
                cm = np.ascontiguousarray(
                    cmask[:, :, c0:c0 + ck].reshape(128, 4 * ck)).view(np.int16)
                gconst[:, 12 * c0 + 8 * ck:12 * (c0 + ck)] = cm

        # unpermute gather indices: canonical slot n reads scratch[rank[n]]
        uidx = np.zeros((16, NUM_REL, 8 * CUNP), dtype=np.int16)
        n = np.arange(SP)
        for r in range(NUM_REL):
            d = per_core[c][r]
            uidx[n % 16, r, n // 16] = d["rank"][n].astype(np.int16)
        uidx = np.tile(uidx, (8, 1, 1))

        in_maps.append({
            "des": des, "tweet": tw, "numcat": ncat,
            "gconst": gconst, "uidx": uidx,
            "wd": wd, "wt": wt, "wnc": wnc, "wsm": wsm,
            "id128": id128, "id32": id32,
        })
    return cfg, in_maps


_CACHE = {}


def kernel(**inputs):
    cfg, in_maps = _prep(inputs)
    key = (tuple(sorted(ABL)),) + tuple(
        (c0, ck) for r in range(NUM_REL) for (c0, ck, _) in cfg.pieces[r])
    if key not in _CACHE:
        _CACHE[key] = build_bass(cfg)
    nc = _CACHE[key]
    res = run_bass_kernel_spmd(nc, in_maps, list(range(N_CORES)))
    outs = []
    for c in range(N_CORES):
        o = res.results[c]["out"]
        outs.append(o.T[0:cfg.shard_real])
    return np.ascontiguousarray(np.concatenate(outs, axis=0).astype(np.float32))


# revision 21
# speedup vs baseline: 2.4223x; 1.7121x over previous
"""BotRGCN on 8 trn2 NeuronCores (SPMD, raw Bacc).

Nodes row-sharded 8 ways (12500/core, padded to 12800). Phase A
(768->8 projections etc.) fully sharded with PE transposes + bf16
matmuls. RGCN layers: AllGather bf16 node features -> shared gather
table viewed as 256B groups of 4 node-rows; per-relation degree-sorted
ELL slot gathers via big SWDGE dma_gather instructions (int16 group
indices); DVE masked select-add (host-precomputed bf16 masks fold the
1/cnt mean scaling); unpermute via one dma_gather from a 256B-row
scratch; dense matmuls in feature-major (^T) space.
"""
import os
import sys
sys.path.insert(0, "/opt/trn_rl_repo")
from contextlib import ExitStack

# ablation flags (timing experiments): comma-separated list in BASS_ABL
ABL = set(os.environ.get("BASS_ABL", "").split(","))

import numpy as np
import ml_dtypes

from concourse import bacc, bass, mybir
from concourse import library_config
from concourse.bass_utils import run_bass_kernel_spmd

F32 = mybir.dt.float32
BF16 = mybir.dt.bfloat16
I16 = mybir.dt.int16
LRELU = mybir.ActivationFunctionType.Lrelu
ACOPY = mybir.ActivationFunctionType.Copy

N_CORES = 8
NUM_REL = 2
CBUF = 48          # gather piece size (ELL columns per dma_gather)
NBUF = 4           # msgs piece buffers (one per SWDGE queue)


class Cfg:
    def __init__(self, shard_real, shard_pad, pieces, n_gcols):
        self.shard_real = shard_real
        self.shard_pad = shard_pad
        self.n_super = shard_pad // 512
        self.nt = shard_pad // 128
        self.c_unp = shard_pad // 128
        self.tabv = N_CORES * shard_pad
        # pieces[r] = list of (col0, ck, adds) ; adds = [(agg_blk0, msg_col0, nblk)]
        self.pieces = pieces
        self.n_gcols = n_gcols


NQ = 4   # SWDGE queues for gather parallelism


def build_bass(cfg: Cfg):
    nc = bacc.Bacc("TRN2", debug=False, num_swdge_queues=NQ)
    mmctx = ExitStack()
    SP = cfg.shard_pad
    NT = cfg.nt
    NS = cfg.n_super
    TABV = cfg.tabv
    CUNP = cfg.c_unp
    NG = cfg.n_gcols

    des_in = nc.declare_dram_parameter("des", [SP, 768], F32, isOutput=False)
    tw_in = nc.declare_dram_parameter("tweet", [SP, 768], F32, isOutput=False)
    ncat_in = nc.declare_dram_parameter("numcat", [SP, 16], F32, isOutput=False)
    gconst_in = nc.declare_dram_parameter("gconst", [128, 12 * NG], I16, isOutput=False)
    uidx_in = nc.declare_dram_parameter("uidx", [128, NUM_REL, 8 * CUNP], I16, isOutput=False)
    wd_in = nc.declare_dram_parameter("wd", [768, 32], BF16, isOutput=False)
    wt_in = nc.declare_dram_parameter("wt", [768, 32], BF16, isOutput=False)
    wnc_in = nc.declare_dram_parameter("wnc", [16, 32], BF16, isOutput=False)
    wsm_in = nc.declare_dram_parameter("wsm", [6, 33, 32], BF16, isOutput=False)
    id128_in = nc.declare_dram_parameter("id128", [128, 128], F32, isOutput=False)
    id32_in = nc.declare_dram_parameter("id32", [32, 32], BF16, isOutput=False)
    out_ext = nc.declare_dram_parameter("out", [2, SP], F32, isOutput=True)

    shard_ag = nc.dram_tensor("shard_ag", [SP, 32], BF16)
    table = nc.dram_tensor("table", [TABV, 32], BF16, addr_space="Shared")
    scratch = [nc.dram_tensor(f"scratch{r}", [SP, 64], F32) for r in range(NUM_REL)]

    live = []

    def sb(name, shape, dt):
        cm = nc.sbuf_tensor(name, shape, dt)
        t = cm.__enter__()
        live.append(cm)
        return t

    def psum(name, shape):
        cm = nc.psum_tensor(name, shape, F32)
        t = cm.__enter__()
        live.append(cm)
        return t

    sb_des = sb("sb_des", [128, 2, 4, 768], F32)
    sb_tw = sb("sb_tw", [128, 2, 4, 768], F32)
    sb_nc = sb("sb_nc", [128, 2, 4, 16], F32)
    sb_desT = sb("sb_desT", [128, 6, 512], BF16)
    sb_twT = sb("sb_twT", [128, 6, 512], BF16)
    sb_ncT = sb("sb_ncT", [16, 512], BF16)
    xT = sb("xT", [33, SP], BF16)
    aggT0 = sb("aggT0", [32, SP], BF16)
    aggT1 = sb("aggT1", [32, SP], BF16)
    agg = sb("agg", [128, NT * 32], F32)
    # gather piece buffers alias the phase-A des/tweet staging (consumed by then)
    _desb = sb_des[:, :, :, :].bitcast(BF16).rearrange("p a b c -> p (a b c)")
    _twb = sb_tw[:, :, :, :].bitcast(BF16).rearrange("p a b c -> p (a b c)")
    _seg = CBUF * 128
    msgs2 = [_desb[:, 0:_seg].rearrange("p (a b) -> p a b", b=128),
             _desb[:, _seg:2 * _seg].rearrange("p (a b) -> p a b", b=128),
             _twb[:, 0:_seg].rearrange("p (a b) -> p a b", b=128),
             _twb[:, _seg:2 * _seg].rearrange("p (a b) -> p a b", b=128)]
    tmp2 = [sb(f"tmp{i}", [128, CBUF, 32], BF16) for i in range(2)]
    sb_gc = sb("sb_gc", [128, NBUF, 12 * CBUF], I16)
    sb_uidx = sb("sb_uidx", [128, NUM_REL, 8 * CUNP], I16)
    unp = sb("unp", [128, CUNP, 64], F32)
    sb_shard = sb("sb_shard", [128, NT, 32], BF16)
    sb_wd = sb("sb_wd", [128, 6, 32], BF16)
    sb_wt = sb("sb_wt", [128, 6, 32], BF16)
    sb_wnc = sb("sb_wnc", [16, 32], BF16)
    sb_wsm = sb("sb_wsm", [33, 6, 32], BF16)
    sb_id128 = sb("sb_id128", [128, 128], F32)
    sb_id32 = sb("sb_id32", [32, 32], BF16)
    sb_x3T = sb("sb_x3T", [33, 512], BF16)
    sb_lg = sb("sb_lg", [2, 2, 512], F32)

    pb = [psum(f"pb{i}", [128, 512]) for i in range(8)]
    pbx = pb[5][:, :].bitcast(BF16)

    table4 = table.ap().rearrange("(g f) d -> g (f d)", f=4)   # [TABV/4, 128]

    plan = {"sync": [], "pe": [], "act": [], "dve": [], "gp": []}

    def op(engine, fn):
        plan[engine].append(fn)

    class Sem:
        def __init__(self, name):
            cm = nc.semaphore(name)
            self.h = cm.__enter__()
            live.append(cm)
            self.n = 0

        def inc(self, inst, k=1):
            inst.then_inc(self.h, k)

        def pinc(self, k=1):
            self.n += k
            return self.n

    s_load = Sem("s_load")
    s_ld = [Sem("s_ld0"), Sem("s_ld1")]
    s_lr = Sem("s_lr")
    s_gq = [Sem(f"s_gq{q}") for q in range(NQ)]   # per-SWDGE-queue gather sems
    s_gc = Sem("s_gc")      # gconst piece loads
    s_tp = Sem("s_tp")
    s_cp = Sem("s_cp")
    s_mm = Sem("s_mm")
    s_x1 = Sem("s_x1")
    s_gp = Sem("s_gp")      # gp-engine bulk DMAs (SWDGE)
    s_cc = Sem("s_cc")
    s_dve = Sem("s_dve")
    s_sh = Sem("s_sh")

    def W(engine, sem, val):
        if val > 0:
            op(engine, lambda eng, s=sem, v=val: eng.wait_ge(s.h, v))

    # ---------------- constants ----------------
    def c_loads(eng):
        eng.dma_start(out=sb_uidx[:], in_=uidx_in[:, :, :]).then_inc(s_load.h, 16)
        eng.dma_start(out=sb_wd[:], in_=wd_in.ap().rearrange("(c p) m -> p c m", p=128)).then_inc(s_load.h, 16)
        eng.dma_start(out=sb_wt[:], in_=wt_in.ap().rearrange("(c p) m -> p c m", p=128)).then_inc(s_load.h, 16)
        eng.dma_start(out=sb_wnc[:], in_=wnc_in[:, :]).then_inc(s_load.h, 16)
        eng.dma_start(out=sb_wsm[:], in_=wsm_in.ap().rearrange("c p m -> p c m")).then_inc(s_load.h, 16)
        eng.dma_start(out=sb_id128[:], in_=id128_in[:, :]).then_inc(s_load.h, 16)
        eng.dma_start(out=sb_id32[:], in_=id32_in[:, :]).then_inc(s_load.h, 16)
    op("sync", c_loads)
    s_load.n += 7 * 16
    NCONST = s_load.n

    op("gp", lambda eng: eng.load_library(library_config.mlp))

    def init_ones(eng):
        eng.memset(xT[32:33, :], 1.0)
        s_dve.inc(eng.memset(sb_x3T[32:33, :], 1.0))
    op("dve", init_ones)
    s_dve.pinc()
    NINIT = s_dve.n

    # =======================================================
    # Phase A
    # =======================================================
    for i in range(NS):
        buf = i % 2
        if i >= 2:
            W("sync", s_mm, 2 * (i - 1))

        def ld(eng, i=i, buf=buf):
            r0 = i * 512
            eng.dma_start(out=sb_des[:, buf, :, :],
                          in_=des_in[r0:r0 + 512, :].rearrange("(t p) c -> p t c", p=128)
                          ).then_inc(s_ld[buf].h, 16)
            eng.dma_start(out=sb_tw[:, buf, :, :],
                          in_=tw_in[r0:r0 + 512, :].rearrange("(t p) c -> p t c", p=128)
                          ).then_inc(s_ld[buf].h, 16)
            eng.dma_start(out=sb_nc[:, buf, :, :],
                          in_=ncat_in[r0:r0 + 512, :].rearrange("(t p) c -> p t c", p=128)
                          ).then_inc(s_ld[buf].h, 16)
        op("sync", ld)
        s_ld[buf].n += 48

        # ---- PE ----
        if i == 0:
            W("pe", s_load, NCONST)
        W("pe", s_ld[buf], 48 * (i // 2 + 1))
        if i >= 1:
            W("pe", s_cp, i * 5)          # ACT copy-rounds of i-1 done (psum WAR)
            W("pe", s_x1, 2 * i)          # act_x1(i-1) consumed pb7

        def pe_nc(eng, buf=buf):
            last = None
            for t in range(4):
                last = nc.tensor.transpose(out=pb[7][0:16, t * 128:(t + 1) * 128],
                                           in_=sb_nc[:, buf, t, :], identity=sb_id128[:])
            s_tp.inc(last)
        op("pe", pe_nc)
        s_tp.pinc()

        for (src, bank0, waitv) in ((sb_des, 0, i * 5 + 2), (sb_tw, 3, i * 5 + 4)):
            if bank0 == 3 and i >= 1:
                W("pe", s_sh, i)          # act_sh(i-1) consumed pb5-alias
            for rnd in range(2):
                if rnd == 1:
                    W("pe", s_cp, waitv)  # ACT copied round 0 of this tensor

                def pe_tp(eng, src=src, bank0=bank0, rnd=rnd, buf=buf):
                    last = None
                    for cc in range(3):
                        c = rnd * 3 + cc
                        for t in range(4):
                            last = nc.tensor.transpose(
                                out=pb[bank0 + cc][:, t * 128:(t + 1) * 128],
                                in_=src[:, buf, t, c * 128:(c + 1) * 128],
                                identity=sb_id128[:])
                    s_tp.inc(last)
                op("pe", pe_tp)
                s_tp.pinc()

        W("pe", s_cp, i * 5 + 5)

        def pe_mm(eng):
            for c in range(6):
                nc.tensor.matmul(pb[6][0:32, :], sb_wd[:, c, :], sb_desT[:, c, :],
                                 start=(c == 0), stop=False)
            for c in range(6):
                nc.tensor.matmul(pb[6][0:32, :], sb_wt[:, c, :], sb_twT[:, c, :],
                                 start=False, stop=False)
            last = nc.tensor.matmul(pb[6][0:32, :], sb_wnc[:, :], sb_ncT[:, :],
                                    start=False, stop=True)
            s_mm.inc(last)
        op("pe", pe_mm)
        s_mm.pinc()

        W("pe", s_x1, 2 * i + 1)
        if i == 0:
            W("pe", s_dve, NINIT)

        def pe_wi(eng, i=i):
            last = nc.tensor.matmul(pb[7][0:32, :], sb_wsm[:, 0, :],
                                    xT[0:33, i * 512:(i + 1) * 512], start=True, stop=True)
            s_mm.inc(last)
        op("pe", pe_wi)
        s_mm.pinc()

        W("pe", s_x1, 2 * i + 2)
        if i >= 1:
            W("pe", s_sh, i)              # act_sh(i-1) consumed pb0
        if i == NS - 1 and cfg.shard_real < SP:
            W("pe", s_dve, NINIT + 1)

        def pe_x1t(eng, i=i):
            last = None
            for t in range(4):
                last = nc.tensor.transpose(
                    out=pbx[:, t * 32:(t + 1) * 32],
                    in_=xT[0:32, i * 512 + t * 128:i * 512 + (t + 1) * 128],
                    identity=sb_id32[:])
            s_tp.inc(last)
        op("pe", pe_x1t)
        s_tp.pinc()

        # ---- ACT ----
        W("act", s_tp, i * 6 + 1)
        op("act", lambda eng: s_cp.inc(eng.activation(out=sb_ncT[:, :], in_=pb[7][0:16, :],
                                                      func=ACOPY)))
        s_cp.pinc()
        k = 0
        for (dstT, bank0) in ((sb_desT, 0), (sb_twT, 3)):
            for rnd in range(2):
                k += 1
                W("act", s_tp, i * 6 + 1 + k)

                def act_cp(eng, dstT=dstT, bank0=bank0, rnd=rnd):
                    last = None
                    for cc in range(3):
                        c = rnd * 3 + cc
                        last = eng.activation(out=dstT[:, c, :], in_=pb[bank0 + cc][:, :],
                                              func=ACOPY)
                    s_cp.inc(last)
                op("act", act_cp)
                s_cp.pinc()

        W("act", s_mm, 2 * i + 1)
        op("act", lambda eng, i=i: s_lr.inc(eng.activation(
            out=xT[0:32, i * 512:(i + 1) * 512], in_=pb[6][0:32, :], func=ACOPY)))
        s_lr.pinc()
        W("dve", s_lr, s_lr.n)

        def act_x(eng, i=i):
            sl = xT[0:32, i * 512:(i + 1) * 512]
            s_x1.inc(nc.vector.scalar_tensor_tensor(
                out=sl, in0=sl, scalar=0.01, in1=sl,
                op0=mybir.AluOpType.mult, op1=mybir.AluOpType.max))
        op("dve", act_x)
        s_x1.pinc()
        W("act", s_mm, 2 * i + 2)
        op("act", lambda eng, i=i: s_lr.inc(eng.activation(
            out=xT[0:32, i * 512:(i + 1) * 512], in_=pb[7][0:32, :], func=ACOPY)))
        s_lr.pinc()
        W("dve", s_lr, s_lr.n)

        def act_x1(eng, i=i):
            sl = xT[0:32, i * 512:(i + 1) * 512]
            s_x1.inc(nc.vector.scalar_tensor_tensor(
                out=sl, in0=sl, scalar=0.01, in1=sl,
                op0=mybir.AluOpType.mult, op1=mybir.AluOpType.max))
        op("dve", act_x1)
        s_x1.pinc()
        if i == NS - 1 and cfg.shard_real < SP:
            W("dve", s_x1, 2 * NS)
            op("dve", lambda eng: s_dve.inc(eng.memset(xT[0:32, cfg.shard_real:SP], 0)))
            s_dve.pinc()
        W("act", s_tp, i * 6 + 6)

        def act_sh(eng, i=i):
            s_sh.inc(eng.activation(
                out=sb_shard[:, 4 * i:4 * i + 4, :].rearrange("p a b -> p (a b)"),
                in_=pbx[:, 0:128], func=ACOPY))
        op("act", act_sh)
        s_sh.pinc()

    # =======================================================
    # RGCN layers
    # =======================================================
    glob = {"gpi": 0}
    piece_dve_after = []   # s_dve count after global piece j's select-adds

    def emit_layer(layer):
        W("gp", s_sh, s_sh.n)
        op("gp", lambda eng: s_gp.inc(
            eng.dma_start(out=shard_ag[:, :].rearrange("(t p) d -> p t d", p=128),
                          in_=sb_shard[:, :, :]), 16))
        s_gp.pinc(16)
        gp_shard_done = s_gp.n
        W("gp", s_gp, gp_shard_done)
        if "noag" not in ABL:
            op("gp", lambda eng: s_cc.inc(eng.collective_compute(
                "AllGather", mybir.AluOpType.bypass,
                ins=[shard_ag[:, :]], outs=[table[:, :]],
                replica_groups=[list(range(N_CORES))])))
            s_cc.pinc()
        CCN = s_cc.n

        for r in range(NUM_REL):
            # memset agg; waits prior relation's scratch write (agg WAR)
            W("dve", s_gp, s_gp.n)
            op("dve", lambda eng: s_dve.inc(eng.memset(agg[:, :], 0)))
            s_dve.pinc()

            for pi, (c0, ck, adds) in enumerate(cfg.pieces[r]):
                if "nogather" in ABL:
                    break
                j = glob["gpi"]
                glob["gpi"] += 1
                pbuf = j % NBUF
                # stream this piece's idx+cmask block
                if j >= NBUF:
                    W("sync", s_dve, piece_dve_after[j - NBUF])
                op("sync", lambda eng, c0=c0, ck=ck, pbuf=pbuf: s_gc.inc(
                    eng.dma_start(out=sb_gc[:, pbuf, 0:12 * ck],
                                  in_=gconst_in[:, 12 * c0:12 * (c0 + ck)]), 16))
                s_gc.pinc(16)
                gcv = s_gc.n
                # gather on gp
                W("gp", s_gc, gcv)
                if pi == 0:
                    W("gp", s_cc, CCN)
                if j >= NBUF:
                    W("gp", s_dve, piece_dve_after[j - NBUF])   # msgs/gc WAR

                q = j % NQ

                def gth(eng, ck=ck, pbuf=pbuf, q=q):
                    s_gq[q].inc(eng.dma_gather(
                        out_ap=msgs2[pbuf][:, 0:ck, :],
                        in_ap=table4,
                        idxs_ap=sb_gc[:, pbuf, 0:8 * ck],
                        num_idxs=128 * ck,
                        num_idxs_reg=128 * ck,
                        elem_size=128,
                        single_packet=False,
                        queue_num=q,
                    ), 16)
                op("gp", gth)
                s_gq[q].pinc(16)
                GQ = s_gq[q].n
                # DVE select-adds
                W("dve", s_gq[q], GQ)

                def sel(eng, ck=ck, adds=adds, pbuf=pbuf):
                    cmbase = 8 * ck
                    gcb = sb_gc[:, pbuf, :].bitcast(BF16)   # [128, 12*CBUF]
                    last = None
                    for b in range(4):
                        tb = tmp2[b % 2]
                        if b > 0:
                            eng.drain()
                        nc.vector.tensor_tensor(
                            out=tb[:, 0:ck, :],
                            in0=msgs2[pbuf][:, 0:ck, 32 * b:32 * b + 32],
                            in1=gcb[:, cmbase + b * ck:cmbase + (b + 1) * ck]
                                .to_broadcast([128, ck, 32]),
                            op=mybir.AluOpType.mult)
                        for (ab, mb, nb) in adds:
                            eng.drain()
                            last = nc.vector.tensor_tensor(
                                out=agg[:, ab * 32:(ab + nb) * 32],
                                in0=agg[:, ab * 32:(ab + nb) * 32],
                                in1=tb[:, mb:mb + nb, :].rearrange("p a b -> p (a b)"),
                                op=mybir.AluOpType.add)
                    s_dve.inc(last)
                op("dve", sel)
                s_dve.pinc()
                piece_dve_after.append(s_dve.n)

            # relation done: agg (sorted order) -> scratch (256B rows)
            W("gp", s_dve, s_dve.n)
            op("gp", lambda eng, r=r: s_gp.inc(
                eng.dma_start(out=scratch[r][:, 0:32].rearrange("(t p) d -> p t d", p=128),
                              in_=agg[:, :].rearrange("p (t d) -> p t d", d=32)), 16))
            s_gp.pinc(16)
            W("gp", s_gp, s_gp.n)
            W("gp", s_tp, s_tp.n)   # prior relation's unp transposes done (unp WAR)

            uq = glob["gpi"] % NQ
            glob["gpi"] += 1
            if "nounp" not in ABL:
                def unp_g(eng, r=r, uq=uq):
                    s_gq[uq].inc(eng.dma_gather(
                        out_ap=unp[:, :, :],
                        in_ap=scratch[r].ap(),
                        idxs_ap=sb_uidx[:, r, :],
                        num_idxs=SP,
                        num_idxs_reg=SP,
                        elem_size=64,
                        single_packet=False,
                        queue_num=uq,
                    ), 16)
                op("gp", unp_g)
                s_gq[uq].pinc(16)
            UNPQ = s_gq[uq].n
            piece_dve_after.append(s_dve.n)

            # transpose unp (canonical node-major fp32) -> aggT (bf16 ^T)
            aggT = aggT0 if r == 0 else aggT1
            W("pe", s_gq[uq], UNPQ)
            C0 = s_cp.n
            T0 = s_tp.n
            for g in range(NT // 4):
                bank = pb[1 + (g % 2)]
                W("pe", s_cp, C0 + g - 1 if g >= 2 else C0)

                def pe_at(eng, g=g, bank=bank):
                    last = None
                    for t in range(4):
                        n = g * 4 + t
                        last = nc.tensor.transpose(out=bank[0:32, t * 128:(t + 1) * 128],
                                                   in_=unp[:, n, 0:32],
                                                   identity=sb_id128[:])
                    s_tp.inc(last)
                op("pe", pe_at)
                s_tp.pinc()
                W("act", s_tp, T0 + g + 1)

                def act_at(eng, g=g, bank=bank, aggT=aggT):
                    s_cp.inc(eng.activation(out=aggT[:, g * 512:(g + 1) * 512],
                                            in_=bank[0:32, :], func=ACOPY))
                op("act", act_at)
                s_cp.pinc()

        # dense tail
        W("pe", s_cp, s_cp.n)
        W("pe", s_x1, s_x1.n)
        X0 = s_x1.n
        for ch in range(NS):
            bank = pb[3 + (ch % 2)]
            if ch >= 2:
                W("pe", s_x1, X0 + ch - 1)

            def pe_tail(eng, ch=ch, bank=bank):
                sl = slice(ch * 512, (ch + 1) * 512)
                nc.tensor.matmul(bank[0:32, :], sb_wsm[:, 1, :], xT[0:33, sl],
                                 start=True, stop=False)
                nc.tensor.matmul(bank[0:32, :], sb_wsm[0:32, 2, :], aggT0[:, sl],
                                 start=False, stop=False)
                last = nc.tensor.matmul(bank[0:32, :], sb_wsm[0:32, 3, :], aggT1[:, sl],
                                        start=False, stop=True)
                s_mm.inc(last)
            op("pe", pe_tail)
            s_mm.pinc()
            W("act", s_mm, s_mm.n)

            def act_tail(eng, ch=ch, bank=bank):
                s_x1.inc(eng.activation(out=xT[0:32, ch * 512:(ch + 1) * 512],
                                        in_=bank[0:32, :], func=ACOPY))
            op("act", act_tail)
            s_x1.pinc()

        if layer == 1:
            S0 = s_sh.n
            X1 = X0
            for ch in range(NS):
                W("pe", s_x1, X1 + ch + 1)
                if ch >= 1:
                    W("pe", s_sh, S0 + ch)
                if ch == NS - 1 and cfg.shard_real < SP:
                    W("dve", s_x1, X1 + NS)
                    op("dve", lambda eng: s_dve.inc(eng.memset(xT[0:32, cfg.shard_real:SP], 0)))
                    s_dve.pinc()
                    W("pe", s_dve, s_dve.n)

                def pe_sh(eng, ch=ch):
                    last = None
                    for t in range(4):
                        last = nc.tensor.transpose(
                            out=pbx[:, t * 32:(t + 1) * 32],
                            in_=xT[0:32, ch * 512 + t * 128:ch * 512 + (t + 1) * 128],
                            identity=sb_id32[:])
                    s_tp.inc(last)
                op("pe", pe_sh)
                s_tp.pinc()
                W("act", s_tp, s_tp.n)
                if ch == 0:
                    W("act", s_gp, gp_shard_done)   # shard DMA of this layer done

                def act_sh2(eng, ch=ch):
                    s_sh.inc(eng.activation(
                        out=sb_shard[:, 4 * ch:4 * ch + 4, :].rearrange("p a b -> p (a b)"),
                        in_=pbx[:, 0:128], func=ACOPY))
                op("act", act_sh2)
                s_sh.pinc()

    emit_layer(1)
    emit_layer(2)

    # =======================================================
    # head
    # =======================================================
    W("pe", s_x1, s_x1.n)
    XH = s_x1.n
    GH = s_gp.n
    for ch in range(NS):
        bank = pb[3 + (ch % 2)]
        if ch >= 1:
            W("pe", s_x1, XH + 2 * ch)

        def pe_h1(eng, ch=ch, bank=bank):
            s_mm.inc(nc.tensor.matmul(bank[0:32, :], sb_wsm[:, 4, :],
                                      xT[0:33, ch * 512:(ch + 1) * 512], start=True, stop=True))
        op("pe", pe_h1)
        s_mm.pinc()
        W("act", s_mm, s_mm.n)
        op("act", lambda eng, bank=bank: s_lr.inc(eng.activation(
            out=sb_x3T[0:32, :], in_=bank[0:32, :], func=ACOPY)))
        s_lr.pinc()
        W("dve", s_lr, s_lr.n)

        def act_h1(eng, ch=ch, bank=bank):
            s_x1.inc(nc.vector.scalar_tensor_tensor(
                out=sb_x3T[0:32, :], in0=sb_x3T[0:32, :], scalar=0.01,
                in1=sb_x3T[0:32, :], op0=mybir.AluOpType.mult, op1=mybir.AluOpType.max))
        op("dve", act_h1)
        s_x1.pinc()
        W("pe", s_x1, s_x1.n)

        def pe_h2(eng, ch=ch, bank=bank):
            s_mm.inc(nc.tensor.matmul(bank[0:2, :], sb_wsm[:, 5, 0:2],
                                      sb_x3T[0:33, :], start=True, stop=True))
        op("pe", pe_h2)
        s_mm.pinc()
        W("act", s_mm, s_mm.n)
        if ch >= 2:
            W("act", s_gp, GH + (ch - 1) * 16)

        def act_h2(eng, ch=ch, bank=bank):
            s_x1.inc(eng.activation(out=sb_lg[:, ch % 2, :], in_=bank[0:2, :],
                                    func=ACOPY))
        op("act", act_h2)
        s_x1.pinc()
        W("gp", s_x1, s_x1.n)

        def gp_out(eng, ch=ch):
            s_gp.inc(eng.dma_start(out=out_ext[:, ch * 512:(ch + 1) * 512],
                                   in_=sb_lg[:, ch % 2, :]), 16)
        op("gp", gp_out)
        s_gp.pinc(16)
    W("gp", s_gp, s_gp.n)

    with nc.Block() as block:
        @block.sync
        def _(eng):
            for f in plan["sync"]:
                f(eng)

        @block.tensor
        def _(eng):
            for f in plan["pe"]:
                f(eng)

        @block.scalar
        def _(eng):
            for f in plan["act"]:
                f(eng)

        @block.vector
        def _(eng):
            for f in plan["dve"]:
                f(eng)

        @block.gpsimd
        def _(eng):
            for f in plan["gp"]:
                f(eng)

    nc.compile()
    nc._live_refs = (live, mmctx)
    return nc


# =======================================================
# Host side
# =======================================================
def _build_structures(edge_index, edge_type, shard_real=12500, shard_pad=12800):
    SP = shard_pad
    src = edge_index[0].astype(np.int64)
    dst = edge_index[1].astype(np.int64)
    et = edge_type.astype(np.int64)
    owner = dst // shard_real
    ldst = dst % shard_real
    trow = (src // shard_real) * SP + (src % shard_real)

    per_core = []
    for c in range(N_CORES):
        rels = []
        for r in range(NUM_REL):
            sel = (owner == c) & (et == r)
            l = ldst[sel]
            t = trow[sel]
            dcnt = np.bincount(l, minlength=SP)
            perm = np.argsort(-dcnt, kind="stable")
            rank = np.empty(SP, dtype=np.int64)
            rank[perm] = np.arange(SP)
            order = np.argsort(rank[l], kind="stable")
            l_s, t_s = l[order], t[order]
            s_sorted = rank[l_s]
            if len(l_s):
                newgrp = np.r_[True, s_sorted[1:] != s_sorted[:-1]]
                gidx = np.cumsum(newgrp) - 1
                starts = np.flatnonzero(newgrp)
                kpos = np.arange(len(l_s)) - starts[gidx]
            else:
                kpos = np.zeros(0, dtype=np.int64)
            maxd = int(dcnt.max()) if len(l) else 0
            Lk = np.array([(dcnt > k).sum() for k in range(maxd)], dtype=np.int64)
            rels.append(dict(dcnt=dcnt, perm=perm, rank=rank, s=s_sorted, k=kpos,
                             t=t_s, maxd=maxd, Lk=Lk))
        per_core.append(rels)

    maxd_g = [max(per_core[c][r]["maxd"] for c in range(N_CORES)) for r in range(NUM_REL)]
    c_r = []
    for r in range(NUM_REL):
        cks = []
        for k in range(maxd_g[r]):
            m = 1
            for c in range(N_CORES):
                Lk = per_core[c][r]["Lk"]
                if k < len(Lk):
                    m = max(m, int(np.ceil(Lk[k] / 128)))
            cks.append(m)
        c_r.append(cks)

    # piece decomposition (shared across cores)
    pieces = []
    colbase_r = []
    gcol = 0
    for r in range(NUM_REL):
        colbase = []
        plist = []
        cur_c0 = gcol
        cur_ck = 0
        cur_adds = []
        for k, ck in enumerate(c_r[r]):
            colbase.append(gcol)
            off = 0
            while off < ck:
                room = CBUF - cur_ck
                if room == 0:
                    plist.append((cur_c0, cur_ck, cur_adds))
                    cur_c0, cur_ck, cur_adds = cur_c0 + CBUF, 0, []
                    room = CBUF
                take = min(room, ck - off)
                cur_adds.append((off, cur_ck, take))
                cur_ck += take
                off += take
            gcol += ck
        if cur_ck:
            plist.append((cur_c0, cur_ck, cur_adds))
        pieces.append(plist)
        colbase_r.append(np.array(colbase, dtype=np.int64))

    cfg = Cfg(shard_real, SP, pieces, gcol)
    return cfg, per_core, colbase_r


def _prep(inputs, shard_real=12500, shard_pad=12800):
    SP = shard_pad
    cfg, per_core, colbase_r = _build_structures(
        inputs["edge_index"], inputs["edge_type"], shard_real, shard_pad)
    NT = cfg.nt
    CUNP = cfg.c_unp
    NG = cfg.n_gcols

    f32 = np.float32
    bf16 = ml_dtypes.bfloat16
    wd = np.zeros((768, 32), dtype=bf16); wd[:, 0:8] = inputs["Wd"].astype(bf16)
    wt = np.zeros((768, 32), dtype=bf16); wt[:, 8:16] = inputs["Wt"].astype(bf16)
    wnc = np.zeros((16, 32), dtype=bf16)
    wnc[0:6, 16:24] = inputs["Wn"].astype(bf16)
    wnc[6:9, 24:32] = inputs["Wc"].astype(bf16)
    bx = np.zeros(32, dtype=np.float32)
    bx[0:8] = inputs["bd"]; bx[8:16] = inputs["bt"]
    bx[16:24] = inputs["bn"]; bx[24:32] = inputs["bc"]
    wnc[9, :] = bx.astype(bf16)
    wsm = np.zeros((6, 33, 32), dtype=bf16)
    wsm[0, 0:32] = inputs["Wi"].astype(bf16)
    wsm[0, 32] = inputs["bi"].astype(bf16)
    wsm[1, 0:32] = inputs["Wroot"].astype(bf16)
    wsm[1, 32] = inputs["brgcn"].astype(bf16)
    wsm[2, 0:32] = inputs["Wrel"][0].astype(bf16)
    wsm[3, 0:32] = inputs["Wrel"][1].astype(bf16)
    wsm[4, 0:32] = inputs["Wo1"].astype(bf16)
    wsm[4, 32] = inputs["bo1"].astype(bf16)
    wsm[5, 0:32, 0:2] = inputs["Wo2"].astype(bf16)
    wsm[5, 32, 0:2] = inputs["bo2"].astype(bf16)
    id128 = np.eye(128, dtype=f32)
    id32 = np.eye(32, dtype=bf16)

    in_maps = []
    for c in range(N_CORES):
        r0, r1 = c * shard_real, (c + 1) * shard_real
        des = np.zeros((SP, 768), dtype=f32); des[0:shard_real] = inputs["des"][r0:r1]
        tw = np.zeros((SP, 768), dtype=f32); tw[0:shard_real] = inputs["tweet"][r0:r1]
        ncat = np.zeros((SP, 16), dtype=f32)
        ncat[0:shard_real, 0:6] = inputs["num_prop"][r0:r1]
        ncat[0:shard_real, 6:9] = inputs["cat_prop"][r0:r1]
        ncat[0:shard_real, 9] = 1.0

        # per-slot gather group indices (16-wrap layout) + 4 bucket masks
        gidx16 = np.zeros((16, 8 * NG), dtype=np.int16)
        cmask = np.zeros((128, 4, NG), dtype=bf16)
        for r in range(NUM_REL):
            d = per_core[c][r]
            if len(d["s"]) == 0:
                continue
            s = d["s"]; t = d["t"]; k = d["k"]
            cols = colbase_r[r][k] + s // 128
            p = s % 128
            cv = (1.0 / np.maximum(d["dcnt"][d["perm"]], 1.0)).astype(f32)
            gidx16[p % 16, 8 * cols + p // 16] = (t // 4).astype(np.int16)
            cmask[p, t % 4, cols] = cv[s].astype(bf16)
        # pack per piece: [8ck i16 idx | 4ck bf16 cmask]
        gconst = np.zeros((128, 12 * NG), dtype=np.int16)
        for r in range(NUM_REL):
            for (c0, ck, _) in cfg.pieces[r]:
                blk = np.tile(gidx16[:, 8 * c0:8 * (c0 + ck)], (8, 1))
                gconst[:, 12 * c0:12 * c0 + 8 * ck] = blk
                cm = np.ascontiguousarray(
                    cmask[:, :, c0:c0 + ck].reshape(128, 4 * ck)).view(np.int16)
                gconst[:, 12 * c0 + 8 * ck:12 * (c0 + ck)] = cm

        # unpermute gather indices: canonical slot n reads scratch[rank[n]]
        uidx = np.zeros((16, NUM_REL, 8 * CUNP), dtype=np.int16)
        n = np.arange(SP)
        for r in range(NUM_REL):
            d = per_core[c][r]
            uidx[n % 16, r, n // 16] = d["rank"][n].astype(np.int16)
        uidx = np.tile(uidx, (8, 1, 1))

        in_maps.append({
            "des": des, "tweet": tw, "numcat": ncat,
            "gconst": gconst, "uidx": uidx,
            "wd": wd, "wt": wt, "wnc": wnc, "wsm": wsm,
            "id128": id128, "id32": id32,
        })
    return cfg, in_maps


_CACHE = {}


def kernel(**inputs):
    cfg, in_maps = _prep(inputs)
    key = (tuple(sorted(ABL)),) + tuple(
        (c0, ck) for r in range(NUM_REL) for (c0, ck, _) in cfg.pieces[r])
    if key not in _CACHE:
        _CACHE[key] = build_bass(cfg)
    nc = _CACHE[key]
    res = run_bass_kernel_spmd(nc, in_maps, list(range(N_CORES)))
    outs = []
    for c in range(N_CORES):
        o = res.results[c]["out"]
        outs.append(o.T[0:cfg.shard_real])
    return np.ascontiguousarray(np.concatenate(outs, axis=0).astype(np.float32))


# revision 23
# speedup vs baseline: 2.5410x; 1.0490x over previous
"""BotRGCN on 8 trn2 NeuronCores (SPMD, raw Bacc).

Nodes row-sharded 8 ways (12500/core, padded to 12800). Phase A
(768->8 projections etc.) fully sharded with PE transposes + bf16
matmuls. RGCN layers: AllGather bf16 node features -> shared gather
table viewed as 256B groups of 4 node-rows; per-relation degree-sorted
ELL slot gathers via big SWDGE dma_gather instructions (int16 group
indices); DVE masked select-add (host-precomputed bf16 masks fold the
1/cnt mean scaling); unpermute via one dma_gather from a 256B-row
scratch; dense matmuls in feature-major (^T) space.
"""
import os
import sys
sys.path.insert(0, "/opt/trn_rl_repo")
from contextlib import ExitStack

# ablation flags (timing experiments): comma-separated list in BASS_ABL
ABL = set(os.environ.get("BASS_ABL", "").split(","))

import numpy as np
import ml_dtypes

from concourse import bacc, bass, mybir
from concourse import library_config
from concourse.bass_utils import run_bass_kernel_spmd

F32 = mybir.dt.float32
BF16 = mybir.dt.bfloat16
I16 = mybir.dt.int16
LRELU = mybir.ActivationFunctionType.Lrelu
ACOPY = mybir.ActivationFunctionType.Copy

N_CORES = 8
NUM_REL = 2
CBUF = 48          # gather piece size (ELL columns per dma_gather)
NBUF = 4           # msgs piece buffers (one per SWDGE queue)


class Cfg:
    def __init__(self, shard_real, shard_pad, pieces, n_gcols):
        self.shard_real = shard_real
        self.shard_pad = shard_pad
        self.n_super = shard_pad // 512
        self.nt = shard_pad // 128
        self.c_unp = shard_pad // 128
        self.tabv = N_CORES * shard_pad
        # pieces[r] = list of (col0, ck, adds) ; adds = [(agg_blk0, msg_col0, nblk)]
        self.pieces = pieces
        self.n_gcols = n_gcols


NQ = 4   # SWDGE queues for gather parallelism


def build_bass(cfg: Cfg):
    nc = bacc.Bacc("TRN2", debug=False, num_swdge_queues=NQ)
    mmctx = ExitStack()
    SP = cfg.shard_pad
    NT = cfg.nt
    NS = cfg.n_super
    TABV = cfg.tabv
    CUNP = cfg.c_unp
    NG = cfg.n_gcols

    des_in = nc.declare_dram_parameter("des", [SP, 768], F32, isOutput=False)
    tw_in = nc.declare_dram_parameter("tweet", [SP, 768], F32, isOutput=False)
    ncat_in = nc.declare_dram_parameter("numcat", [SP, 16], F32, isOutput=False)
    gconst_in = nc.declare_dram_parameter("gconst", [128, 12 * NG], I16, isOutput=False)
    uidx_in = nc.declare_dram_parameter("uidx", [128, NUM_REL, 8 * CUNP], I16, isOutput=False)
    wd_in = nc.declare_dram_parameter("wd", [768, 32], BF16, isOutput=False)
    wt_in = nc.declare_dram_parameter("wt", [768, 32], BF16, isOutput=False)
    wnc_in = nc.declare_dram_parameter("wnc", [16, 32], BF16, isOutput=False)
    wsm_in = nc.declare_dram_parameter("wsm", [6, 33, 32], BF16, isOutput=False)
    id128_in = nc.declare_dram_parameter("id128", [128, 128], F32, isOutput=False)
    id32_in = nc.declare_dram_parameter("id32", [32, 32], BF16, isOutput=False)
    out_ext = nc.declare_dram_parameter("out", [2, SP], F32, isOutput=True)

    shard_ag = nc.dram_tensor("shard_ag", [SP, 32], BF16)
    table = nc.dram_tensor("table", [TABV, 32], BF16, addr_space="Shared")
    scratch = [nc.dram_tensor(f"scratch{r}", [SP, 64], F32) for r in range(NUM_REL)]

    live = []

    def sb(name, shape, dt):
        cm = nc.sbuf_tensor(name, shape, dt)
        t = cm.__enter__()
        live.append(cm)
        return t

    def psum(name, shape):
        cm = nc.psum_tensor(name, shape, F32)
        t = cm.__enter__()
        live.append(cm)
        return t

    sb_des = sb("sb_des", [128, 2, 4, 768], F32)
    sb_tw = sb("sb_tw", [128, 2, 4, 768], F32)
    sb_nc = sb("sb_nc", [128, 2, 4, 16], F32)
    sb_desT = sb("sb_desT", [128, 6, 512], BF16)
    sb_twT = sb("sb_twT", [128, 6, 512], BF16)
    sb_ncT = sb("sb_ncT", [16, 512], BF16)
    xT = sb("xT", [33, SP], BF16)
    aggT0 = sb("aggT0", [32, SP], BF16)
    aggT1 = sb("aggT1", [32, SP], BF16)
    agg = sb("agg", [128, NT * 32], F32)
    # gather piece buffers alias the phase-A des/tweet staging (consumed by then)
    _desb = sb_des[:, :, :, :].bitcast(BF16).rearrange("p a b c -> p (a b c)")
    _twb = sb_tw[:, :, :, :].bitcast(BF16).rearrange("p a b c -> p (a b c)")
    _seg = CBUF * 128
    msgs2 = [_desb[:, 0:_seg].rearrange("p (a b) -> p a b", b=128),
             _desb[:, _seg:2 * _seg].rearrange("p (a b) -> p a b", b=128),
             _twb[:, 0:_seg].rearrange("p (a b) -> p a b", b=128),
             _twb[:, _seg:2 * _seg].rearrange("p (a b) -> p a b", b=128)]
    tmp2 = [sb(f"tmp{i}", [128, CBUF, 32], BF16) for i in range(2)]
    sb_gc = sb("sb_gc", [128, NBUF, 12 * CBUF], I16)
    sb_uidx = sb("sb_uidx", [128, NUM_REL, 8 * CUNP], I16)
    unp = sb("unp", [128, CUNP, 64], F32)
    sb_shard = sb("sb_shard", [128, NT, 32], BF16)
    sb_wd = sb("sb_wd", [128, 6, 32], BF16)
    sb_wt = sb("sb_wt", [128, 6, 32], BF16)
    sb_wnc = sb("sb_wnc", [16, 32], BF16)
    sb_wsm = sb("sb_wsm", [33, 6, 32], BF16)
    sb_id128 = sb("sb_id128", [128, 128], F32)
    sb_id32 = sb("sb_id32", [32, 32], BF16)
    sb_x3T = sb("sb_x3T", [33, 512], BF16)
    sb_lg = sb("sb_lg", [2, 2, 512], F32)

    pb = [psum(f"pb{i}", [128, 512]) for i in range(8)]
    pbx = pb[5][:, :].bitcast(BF16)

    table4 = table.ap().rearrange("(g f) d -> g (f d)", f=4)   # [TABV/4, 128]

    plan = {"sync": [], "pe": [], "act": [], "dve": [], "gp": []}

    def op(engine, fn):
        plan[engine].append(fn)

    class Sem:
        def __init__(self, name):
            cm = nc.semaphore(name)
            self.h = cm.__enter__()
            live.append(cm)
            self.n = 0

        def inc(self, inst, k=1):
            inst.then_inc(self.h, k)

        def pinc(self, k=1):
            self.n += k
            return self.n

    s_load = Sem("s_load")
    s_ld = [Sem("s_ld0"), Sem("s_ld1")]
    s_lr = Sem("s_lr")
    s_gq = [Sem(f"s_gq{q}") for q in range(NQ)]   # per-SWDGE-queue gather sems
    s_gc = Sem("s_gc")      # gconst piece loads
    s_tp = Sem("s_tp")
    s_cp = Sem("s_cp")
    s_mm = Sem("s_mm")
    s_x1 = Sem("s_x1")
    s_gp = Sem("s_gp")      # gp-engine bulk DMAs (SWDGE)
    s_cc = Sem("s_cc")
    s_dve = Sem("s_dve")
    s_sh = Sem("s_sh")

    def W(engine, sem, val):
        if val > 0:
            op(engine, lambda eng, s=sem, v=val: eng.wait_ge(s.h, v))

    # ---------------- constants ----------------
    def c_loads(eng):
        eng.dma_start(out=sb_uidx[:], in_=uidx_in[:, :, :]).then_inc(s_load.h, 16)
        eng.dma_start(out=sb_wd[:], in_=wd_in.ap().rearrange("(c p) m -> p c m", p=128)).then_inc(s_load.h, 16)
        eng.dma_start(out=sb_wt[:], in_=wt_in.ap().rearrange("(c p) m -> p c m", p=128)).then_inc(s_load.h, 16)
        eng.dma_start(out=sb_wnc[:], in_=wnc_in[:, :]).then_inc(s_load.h, 16)
        eng.dma_start(out=sb_wsm[:], in_=wsm_in.ap().rearrange("c p m -> p c m")).then_inc(s_load.h, 16)
        eng.dma_start(out=sb_id128[:], in_=id128_in[:, :]).then_inc(s_load.h, 16)
        eng.dma_start(out=sb_id32[:], in_=id32_in[:, :]).then_inc(s_load.h, 16)
    op("sync", c_loads)
    s_load.n += 7 * 16
    NCONST = s_load.n

    op("gp", lambda eng: eng.load_library(library_config.mlp))

    def init_ones(eng):
        eng.memset(xT[32:33, :], 1.0)
        s_dve.inc(eng.memset(sb_x3T[32:33, :], 1.0))
    op("dve", init_ones)
    s_dve.pinc()
    NINIT = s_dve.n

    # =======================================================
    # Phase A
    # =======================================================
    for i in range(NS):
        buf = i % 2
        if i >= 2:
            W("sync", s_mm, 2 * (i - 1))

        def ld(eng, i=i, buf=buf):
            r0 = i * 512
            eng.dma_start(out=sb_des[:, buf, :, :],
                          in_=des_in[r0:r0 + 512, :].rearrange("(t p) c -> p t c", p=128)
                          ).then_inc(s_ld[buf].h, 16)
            eng.dma_start(out=sb_tw[:, buf, :, :],
                          in_=tw_in[r0:r0 + 512, :].rearrange("(t p) c -> p t c", p=128)
                          ).then_inc(s_ld[buf].h, 16)
            eng.dma_start(out=sb_nc[:, buf, :, :],
                          in_=ncat_in[r0:r0 + 512, :].rearrange("(t p) c -> p t c", p=128)
                          ).then_inc(s_ld[buf].h, 16)
        op("sync", ld)
        s_ld[buf].n += 48

        # ---- PE ----
        if i == 0:
            W("pe", s_load, NCONST)
        W("pe", s_ld[buf], 48 * (i // 2 + 1))
        if i >= 1:
            W("pe", s_cp, i * 5)          # ACT copy-rounds of i-1 done (psum WAR)
            W("pe", s_x1, 2 * i)          # act_x1(i-1) consumed pb7

        def pe_nc(eng, buf=buf):
            last = None
            for t in range(4):
                last = nc.tensor.transpose(out=pb[7][0:16, t * 128:(t + 1) * 128],
                                           in_=sb_nc[:, buf, t, :], identity=sb_id128[:])
            s_tp.inc(last)
        op("pe", pe_nc)
        s_tp.pinc()

        for (src, bank0, waitv) in ((sb_des, 0, i * 5 + 2), (sb_tw, 3, i * 5 + 4)):
            if bank0 == 3 and i >= 1:
                W("pe", s_sh, i)          # act_sh(i-1) consumed pb5-alias
            for rnd in range(2):
                if rnd == 1:
                    W("pe", s_cp, waitv)  # ACT copied round 0 of this tensor

                def pe_tp(eng, src=src, bank0=bank0, rnd=rnd, buf=buf):
                    last = None
                    for cc in range(3):
                        c = rnd * 3 + cc
                        for t in range(4):
                            last = nc.tensor.transpose(
                                out=pb[bank0 + cc][:, t * 128:(t + 1) * 128],
                                in_=src[:, buf, t, c * 128:(c + 1) * 128],
                                identity=sb_id128[:])
                    s_tp.inc(last)
                op("pe", pe_tp)
                s_tp.pinc()

        W("pe", s_cp, i * 5 + 5)

        def pe_mm(eng):
            for c in range(6):
                nc.tensor.matmul(pb[6][0:32, :], sb_wd[:, c, :], sb_desT[:, c, :],
                                 start=(c == 0), stop=False)
            for c in range(6):
                nc.tensor.matmul(pb[6][0:32, :], sb_wt[:, c, :], sb_twT[:, c, :],
                                 start=False, stop=False)
            last = nc.tensor.matmul(pb[6][0:32, :], sb_wnc[:, :], sb_ncT[:, :],
                                    start=False, stop=True)
            s_mm.inc(last)
        op("pe", pe_mm)
        s_mm.pinc()

        W("pe", s_x1, 2 * i + 1)
        if i == 0:
            W("pe", s_dve, NINIT)

        def pe_wi(eng, i=i):
            last = nc.tensor.matmul(pb[7][0:32, :], sb_wsm[:, 0, :],
                                    xT[0:33, i * 512:(i + 1) * 512], start=True, stop=True)
            s_mm.inc(last)
        op("pe", pe_wi)
        s_mm.pinc()

        W("pe", s_x1, 2 * i + 2)
        if i >= 1:
            W("pe", s_sh, i)              # act_sh(i-1) consumed pb0
        if i == NS - 1 and cfg.shard_real < SP:
            W("pe", s_dve, NINIT + 1)

        def pe_x1t(eng, i=i):
            last = None
            for t in range(4):
                last = nc.tensor.transpose(
                    out=pbx[:, t * 32:(t + 1) * 32],
                    in_=xT[0:32, i * 512 + t * 128:i * 512 + (t + 1) * 128],
                    identity=sb_id32[:])
            s_tp.inc(last)
        op("pe", pe_x1t)
        s_tp.pinc()

        # ---- ACT ----
        W("act", s_tp, i * 6 + 1)
        op("act", lambda eng: s_cp.inc(eng.activation(out=sb_ncT[:, :], in_=pb[7][0:16, :],
                                                      func=ACOPY)))
        s_cp.pinc()
        k = 0
        for (dstT, bank0) in ((sb_desT, 0), (sb_twT, 3)):
            for rnd in range(2):
                k += 1
                W("act", s_tp, i * 6 + 1 + k)

                def act_cp(eng, dstT=dstT, bank0=bank0, rnd=rnd):
                    last = None
                    for cc in range(3):
                        c = rnd * 3 + cc
                        last = eng.activation(out=dstT[:, c, :], in_=pb[bank0 + cc][:, :],
                                              func=ACOPY)
                    s_cp.inc(last)
                op("act", act_cp)
                s_cp.pinc()

        W("act", s_mm, 2 * i + 1)
        op("act", lambda eng, i=i: s_lr.inc(eng.activation(
            out=xT[0:32, i * 512:(i + 1) * 512], in_=pb[6][0:32, :], func=ACOPY)))
        s_lr.pinc()
        W("dve", s_lr, s_lr.n)

        def act_x(eng, i=i):
            sl = xT[0:32, i * 512:(i + 1) * 512]
            s_x1.inc(nc.vector.scalar_tensor_tensor(
                out=sl, in0=sl, scalar=0.01, in1=sl,
                op0=mybir.AluOpType.mult, op1=mybir.AluOpType.max))
        op("dve", act_x)
        s_x1.pinc()
        W("act", s_mm, 2 * i + 2)
        op("act", lambda eng, i=i: s_lr.inc(eng.activation(
            out=xT[0:32, i * 512:(i + 1) * 512], in_=pb[7][0:32, :], func=ACOPY)))
        s_lr.pinc()
        W("dve", s_lr, s_lr.n)

        def act_x1(eng, i=i):
            sl = xT[0:32, i * 512:(i + 1) * 512]
            s_x1.inc(nc.vector.scalar_tensor_tensor(
                out=sl, in0=sl, scalar=0.01, in1=sl,
                op0=mybir.AluOpType.mult, op1=mybir.AluOpType.max))
        op("dve", act_x1)
        s_x1.pinc()
        if i == NS - 1 and cfg.shard_real < SP:
            W("dve", s_x1, 2 * NS)
            op("dve", lambda eng: s_dve.inc(eng.memset(xT[0:32, cfg.shard_real:SP], 0)))
            s_dve.pinc()
        W("act", s_tp, i * 6 + 6)

        def act_sh(eng, i=i):
            s_sh.inc(eng.activation(
                out=sb_shard[:, 4 * i:4 * i + 4, :].rearrange("p a b -> p (a b)"),
                in_=pbx[:, 0:128], func=ACOPY))
        op("act", act_sh)
        s_sh.pinc()

    # =======================================================
    # RGCN layers
    # =======================================================
    glob = {"gpi": 0}
    piece_dve_after = []   # s_dve count after global piece j's select-adds

    def emit_layer(layer):
        HSP = SP // 2
        S_pre = s_sh.n - NS    # act_sh of this layer's x starts at S_pre+1
        for cag in range(2):
            # chunk 0 ready after 13 shard writes (rows 0..6656), chunk 1 after all
            W("gp", s_sh, S_pre + (13 if cag == 0 else NS))
            op("gp", lambda eng, cag=cag: s_gp.inc(
                eng.dma_start(
                    out=shard_ag[cag * HSP:(cag + 1) * HSP, :]
                        .rearrange("(t p) d -> p t d", p=128),
                    in_=sb_shard[:, cag * (NT // 2):(cag + 1) * (NT // 2), :]), 16))
            s_gp.pinc(16)
            W("gp", s_gp, s_gp.n)
            if "noag" not in ABL:
                op("gp", lambda eng, cag=cag: s_cc.inc(eng.collective_compute(
                    "AllGather", mybir.AluOpType.bypass,
                    ins=[shard_ag[cag * HSP:(cag + 1) * HSP, :]],
                    outs=[table[cag * N_CORES * HSP:(cag + 1) * N_CORES * HSP, :]],
                    replica_groups=[list(range(N_CORES))])))
                s_cc.pinc()
        gp_shard_done = s_gp.n
        CCN = s_cc.n

        for r in range(NUM_REL):
            # memset agg; waits prior relation's scratch write (agg WAR)
            W("dve", s_gp, s_gp.n)
            op("dve", lambda eng: s_dve.inc(eng.memset(agg[:, :], 0)))
            s_dve.pinc()

            for pi, (c0, ck, adds) in enumerate(cfg.pieces[r]):
                if "nogather" in ABL:
                    break
                j = glob["gpi"]
                glob["gpi"] += 1
                pbuf = j % NBUF
                # stream this piece's idx+cmask block
                if j >= NBUF:
                    W("sync", s_dve, piece_dve_after[j - NBUF])
                op("sync", lambda eng, c0=c0, ck=ck, pbuf=pbuf: s_gc.inc(
                    eng.dma_start(out=sb_gc[:, pbuf, 0:12 * ck],
                                  in_=gconst_in[:, 12 * c0:12 * (c0 + ck)]), 16))
                s_gc.pinc(16)
                gcv = s_gc.n
                # gather on gp
                W("gp", s_gc, gcv)
                if pi == 0:
                    W("gp", s_cc, CCN)
                if j >= NBUF:
                    W("gp", s_dve, piece_dve_after[j - NBUF])   # msgs/gc WAR

                q = j % NQ

                def gth(eng, ck=ck, pbuf=pbuf, q=q):
                    s_gq[q].inc(eng.dma_gather(
                        out_ap=msgs2[pbuf][:, 0:ck, :],
                        in_ap=table4,
                        idxs_ap=sb_gc[:, pbuf, 0:8 * ck],
                        num_idxs=128 * ck,
                        num_idxs_reg=128 * ck,
                        elem_size=128,
                        single_packet=False,
                        queue_num=q,
                    ), 16)
                op("gp", gth)
                s_gq[q].pinc(16)
                GQ = s_gq[q].n
                # DVE select-adds
                W("dve", s_gq[q], GQ)

                def sel(eng, ck=ck, adds=adds, pbuf=pbuf):
                    cmbase = 8 * ck
                    gcb = sb_gc[:, pbuf, :].bitcast(BF16)   # [128, 12*CBUF]
                    last = None
                    for b in range(4):
                        tb = tmp2[b % 2]
                        if b > 0:
                            eng.drain()
                        nc.vector.tensor_tensor(
                            out=tb[:, 0:ck, :],
                            in0=msgs2[pbuf][:, 0:ck, 32 * b:32 * b + 32],
                            in1=gcb[:, cmbase + b * ck:cmbase + (b + 1) * ck]
                                .to_broadcast([128, ck, 32]),
                            op=mybir.AluOpType.mult)
                        for (ab, mb, nb) in adds:
                            eng.drain()
                            last = nc.vector.tensor_tensor(
                                out=agg[:, ab * 32:(ab + nb) * 32],
                                in0=agg[:, ab * 32:(ab + nb) * 32],
                                in1=tb[:, mb:mb + nb, :].rearrange("p a b -> p (a b)"),
                                op=mybir.AluOpType.add)
                    s_dve.inc(last)
                op("dve", sel)
                s_dve.pinc()
                piece_dve_after.append(s_dve.n)

            # relation done: agg (sorted order) -> scratch (256B rows)
            W("gp", s_dve, s_dve.n)
            op("gp", lambda eng, r=r: s_gp.inc(
                eng.dma_start(out=scratch[r][:, 0:32].rearrange("(t p) d -> p t d", p=128),
                              in_=agg[:, :].rearrange("p (t d) -> p t d", d=32)), 16))
            s_gp.pinc(16)
            W("gp", s_gp, s_gp.n)
            W("gp", s_tp, s_tp.n)   # prior relation's unp transposes done (unp WAR)

            UCH = SP // NQ          # 3200 idx per chunk
            UCB = CUNP // NQ        # 25 out blocks per chunk
            uql = []
            for c4 in range(NQ):
                uq = glob["gpi"] % NQ
                glob["gpi"] += 1
                piece_dve_after.append(s_dve.n)
                if "nounp" not in ABL:
                    def unp_g(eng, r=r, uq=uq, c4=c4):
                        s_gq[uq].inc(eng.dma_gather(
                            out_ap=unp[:, c4 * UCB:(c4 + 1) * UCB, :],
                            in_ap=scratch[r].ap(),
                            idxs_ap=sb_uidx[:, r, c4 * (UCH // 16):(c4 + 1) * (UCH // 16)],
                            num_idxs=UCH,
                            num_idxs_reg=UCH,
                            elem_size=64,
                            single_packet=False,
                            queue_num=uq,
                        ), 16)
                    op("gp", unp_g)
                    s_gq[uq].pinc(16)
                uql.append((uq, s_gq[uq].n))

            # transpose unp (canonical node-major fp32) -> aggT (bf16 ^T)
            aggT = aggT0 if r == 0 else aggT1
            for (uq, v) in uql:
                W("pe", s_gq[uq], v)
            C0 = s_cp.n
            T0 = s_tp.n
            for g in range(NT // 4):
                bank = pb[1 + (g % 2)]
                W("pe", s_cp, C0 + g - 1 if g >= 2 else C0)

                def pe_at(eng, g=g, bank=bank):
                    last = None
                    for t in range(4):
                        n = g * 4 + t
                        last = nc.tensor.transpose(out=bank[0:32, t * 128:(t + 1) * 128],
                                                   in_=unp[:, n, 0:32],
                                                   identity=sb_id128[:])
                    s_tp.inc(last)
                op("pe", pe_at)
                s_tp.pinc()
                W("act", s_tp, T0 + g + 1)

                def act_at(eng, g=g, bank=bank, aggT=aggT):
                    s_cp.inc(eng.activation(out=aggT[:, g * 512:(g + 1) * 512],
                                            in_=bank[0:32, :], func=ACOPY))
                op("act", act_at)
                s_cp.pinc()

        # dense tail
        W("pe", s_cp, s_cp.n)
        W("pe", s_x1, s_x1.n)
        X0 = s_x1.n
        for ch in range(NS):
            bank = pb[3 + (ch % 2)]
            if ch >= 2:
                W("pe", s_x1, X0 + ch - 1)

            def pe_tail(eng, ch=ch, bank=bank):
                sl = slice(ch * 512, (ch + 1) * 512)
                nc.tensor.matmul(bank[0:32, :], sb_wsm[:, 1, :], xT[0:33, sl],
                                 start=True, stop=False)
                nc.tensor.matmul(bank[0:32, :], sb_wsm[0:32, 2, :], aggT0[:, sl],
                                 start=False, stop=False)
                last = nc.tensor.matmul(bank[0:32, :], sb_wsm[0:32, 3, :], aggT1[:, sl],
                                        start=False, stop=True)
                s_mm.inc(last)
            op("pe", pe_tail)
            s_mm.pinc()
            W("act", s_mm, s_mm.n)

            def act_tail(eng, ch=ch, bank=bank):
                s_x1.inc(eng.activation(out=xT[0:32, ch * 512:(ch + 1) * 512],
                                        in_=bank[0:32, :], func=ACOPY))
            op("act", act_tail)
            s_x1.pinc()

        if layer == 1:
            S0 = s_sh.n
            X1 = X0
            for ch in range(NS):
                W("pe", s_x1, X1 + ch + 1)
                if ch >= 1:
                    W("pe", s_sh, S0 + ch)
                if ch == NS - 1 and cfg.shard_real < SP:
                    W("dve", s_x1, X1 + NS)
                    op("dve", lambda eng: s_dve.inc(eng.memset(xT[0:32, cfg.shard_real:SP], 0)))
                    s_dve.pinc()
                    W("pe", s_dve, s_dve.n)

                def pe_sh(eng, ch=ch):
                    last = None
                    for t in range(4):
                        last = nc.tensor.transpose(
                            out=pbx[:, t * 32:(t + 1) * 32],
                            in_=xT[0:32, ch * 512 + t * 128:ch * 512 + (t + 1) * 128],
                            identity=sb_id32[:])
                    s_tp.inc(last)
                op("pe", pe_sh)
                s_tp.pinc()
                W("act", s_tp, s_tp.n)
                if ch == 0:
                    W("act", s_gp, gp_shard_done)   # shard DMA of this layer done

                def act_sh2(eng, ch=ch):
                    s_sh.inc(eng.activation(
                        out=sb_shard[:, 4 * ch:4 * ch + 4, :].rearrange("p a b -> p (a b)"),
                        in_=pbx[:, 0:128], func=ACOPY))
                op("act", act_sh2)
                s_sh.pinc()

    emit_layer(1)
    emit_layer(2)

    # =======================================================
    # head
    # =======================================================
    W("pe", s_x1, s_x1.n)
    XH = s_x1.n
    GH = s_gp.n
    for ch in range(NS):
        bank = pb[3 + (ch % 2)]
        if ch >= 1:
            W("pe", s_x1, XH + 2 * ch)

        def pe_h1(eng, ch=ch, bank=bank):
            s_mm.inc(nc.tensor.matmul(bank[0:32, :], sb_wsm[:, 4, :],
                                      xT[0:33, ch * 512:(ch + 1) * 512], start=True, stop=True))
        op("pe", pe_h1)
        s_mm.pinc()
        W("act", s_mm, s_mm.n)
        op("act", lambda eng, bank=bank: s_lr.inc(eng.activation(
            out=sb_x3T[0:32, :], in_=bank[0:32, :], func=ACOPY)))
        s_lr.pinc()
        W("dve", s_lr, s_lr.n)

        def act_h1(eng, ch=ch, bank=bank):
            s_x1.inc(nc.vector.scalar_tensor_tensor(
                out=sb_x3T[0:32, :], in0=sb_x3T[0:32, :], scalar=0.01,
                in1=sb_x3T[0:32, :], op0=mybir.AluOpType.mult, op1=mybir.AluOpType.max))
        op("dve", act_h1)
        s_x1.pinc()
        W("pe", s_x1, s_x1.n)

        def pe_h2(eng, ch=ch, bank=bank):
            s_mm.inc(nc.tensor.matmul(bank[0:2, :], sb_wsm[:, 5, 0:2],
                                      sb_x3T[0:33, :], start=True, stop=True))
        op("pe", pe_h2)
        s_mm.pinc()
        W("act", s_mm, s_mm.n)
        if ch >= 2:
            W("act", s_gp, GH + (ch - 1) * 16)

        def act_h2(eng, ch=ch, bank=bank):
            s_x1.inc(eng.activation(out=sb_lg[:, ch % 2, :], in_=bank[0:2, :],
                                    func=ACOPY))
        op("act", act_h2)
        s_x1.pinc()
        W("gp", s_x1, s_x1.n)

        def gp_out(eng, ch=ch):
            s_gp.inc(eng.dma_start(out=out_ext[:, ch * 512:(ch + 1) * 512],
                                   in_=sb_lg[:, ch % 2, :]), 16)
        op("gp", gp_out)
        s_gp.pinc(16)
    W("gp", s_gp, s_gp.n)

    with nc.Block() as block:
        @block.sync
        def _(eng):
            for f in plan["sync"]:
                f(eng)

        @block.tensor
        def _(eng):
            for f in plan["pe"]:
                f(eng)

        @block.scalar
        def _(eng):
            for f in plan["act"]:
                f(eng)

        @block.vector
        def _(eng):
            for f in plan["dve"]:
                f(eng)

        @block.gpsimd
        def _(eng):
            for f in plan["gp"]:
                f(eng)

    nc.compile()
    nc._live_refs = (live, mmctx)
    return nc


# =======================================================
# Host side
# =======================================================
def _build_structures(edge_index, edge_type, shard_real=12500, shard_pad=12800):
    SP = shard_pad
    src = edge_index[0].astype(np.int64)
    dst = edge_index[1].astype(np.int64)
    et = edge_type.astype(np.int64)
    owner = dst // shard_real
    ldst = dst % shard_real
    # chunk-major table layout (AllGather runs in 2 chunks of SP/2 rows):
    # row of (core k, local n) = (n // (SP/2)) * (8*SP/2) + k * (SP/2) + n % (SP/2)
    _hsp = shard_pad // 2
    _srck = src // shard_real
    _srcn = src % shard_real
    trow = (_srcn // _hsp) * (N_CORES * _hsp) + _srck * _hsp + (_srcn % _hsp)

    per_core = []
    for c in range(N_CORES):
        rels = []
        for r in range(NUM_REL):
            sel = (owner == c) & (et == r)
            l = ldst[sel]
            t = trow[sel]
            dcnt = np.bincount(l, minlength=SP)
            perm = np.argsort(-dcnt, kind="stable")
            rank = np.empty(SP, dtype=np.int64)
            rank[perm] = np.arange(SP)
            order = np.argsort(rank[l], kind="stable")
            l_s, t_s = l[order], t[order]
            s_sorted = rank[l_s]
            if len(l_s):
                newgrp = np.r_[True, s_sorted[1:] != s_sorted[:-1]]
                gidx = np.cumsum(newgrp) - 1
                starts = np.flatnonzero(newgrp)
                kpos = np.arange(len(l_s)) - starts[gidx]
            else:
                kpos = np.zeros(0, dtype=np.int64)
            maxd = int(dcnt.max()) if len(l) else 0
            Lk = np.array([(dcnt > k).sum() for k in range(maxd)], dtype=np.int64)
            rels.append(dict(dcnt=dcnt, perm=perm, rank=rank, s=s_sorted, k=kpos,
                             t=t_s, maxd=maxd, Lk=Lk))
        per_core.append(rels)

    maxd_g = [max(per_core[c][r]["maxd"] for c in range(N_CORES)) for r in range(NUM_REL)]
    c_r = []
    for r in range(NUM_REL):
        cks = []
        for k in range(maxd_g[r]):
            m = 1
            for c in range(N_CORES):
                Lk = per_core[c][r]["Lk"]
                if k < len(Lk):
                    m = max(m, int(np.ceil(Lk[k] / 128)))
            cks.append(m)
        c_r.append(cks)

    # piece decomposition (shared across cores)
    pieces = []
    colbase_r = []
    gcol = 0
    for r in range(NUM_REL):
        colbase = []
        plist = []
        cur_c0 = gcol
        cur_ck = 0
        cur_adds = []
        for k, ck in enumerate(c_r[r]):
            colbase.append(gcol)
            off = 0
            while off < ck:
                room = CBUF - cur_ck
                if room == 0:
                    plist.append((cur_c0, cur_ck, cur_adds))
                    cur_c0, cur_ck, cur_adds = cur_c0 + CBUF, 0, []
                    room = CBUF
                take = min(room, ck - off)
                cur_adds.append((off, cur_ck, take))
                cur_ck += take
                off += take
            gcol += ck
        if cur_ck:
            plist.append((cur_c0, cur_ck, cur_adds))
        pieces.append(plist)
        colbase_r.append(np.array(colbase, dtype=np.int64))

    cfg = Cfg(shard_real, SP, pieces, gcol)
    return cfg, per_core, colbase_r


def _prep(inputs, shard_real=12500, shard_pad=12800):
    SP = shard_pad
    cfg, per_core, colbase_r = _build_structures(
        inputs["edge_index"], inputs["edge_type"], shard_real, shard_pad)
    NT = cfg.nt
    CUNP = cfg.c_unp
    NG = cfg.n_gcols

    f32 = np.float32
    bf16 = ml_dtypes.bfloat16
    wd = np.zeros((768, 32), dtype=bf16); wd[:, 0:8] = inputs["Wd"].astype(bf16)
    wt = np.zeros((768, 32), dtype=bf16); wt[:, 8:16] = inputs["Wt"].astype(bf16)
    wnc = np.zeros((16, 32), dtype=bf16)
    wnc[0:6, 16:24] = inputs["Wn"].astype(bf16)
    wnc[6:9, 24:32] = inputs["Wc"].astype(bf16)
    bx = np.zeros(32, dtype=np.float32)
    bx[0:8] = inputs["bd"]; bx[8:16] = inputs["bt"]
    bx[16:24] = inputs["bn"]; bx[24:32] = inputs["bc"]
    wnc[9, :] = bx.astype(bf16)
    wsm = np.zeros((6, 33, 32), dtype=bf16)
    wsm[0, 0:32] = inputs["Wi"].astype(bf16)
    wsm[0, 32] = inputs["bi"].astype(bf16)
    wsm[1, 0:32] = inputs["Wroot"].astype(bf16)
    wsm[1, 32] = inputs["brgcn"].astype(bf16)
    wsm[2, 0:32] = inputs["Wrel"][0].astype(bf16)
    wsm[3, 0:32] = inputs["Wrel"][1].astype(bf16)
    wsm[4, 0:32] = inputs["Wo1"].astype(bf16)
    wsm[4, 32] = inputs["bo1"].astype(bf16)
    wsm[5, 0:32, 0:2] = inputs["Wo2"].astype(bf16)
    wsm[5, 32, 0:2] = inputs["bo2"].astype(bf16)
    id128 = np.eye(128, dtype=f32)
    id32 = np.eye(32, dtype=bf16)

    in_maps = []
    for c in range(N_CORES):
        r0, r1 = c * shard_real, (c + 1) * shard_real
        des = np.zeros((SP, 768), dtype=f32); des[0:shard_real] = inputs["des"][r0:r1]
        tw = np.zeros((SP, 768), dtype=f32); tw[0:shard_real] = inputs["tweet"][r0:r1]
        ncat = np.zeros((SP, 16), dtype=f32)
        ncat[0:shard_real, 0:6] = inputs["num_prop"][r0:r1]
        ncat[0:shard_real, 6:9] = inputs["cat_prop"][r0:r1]
        ncat[0:shard_real, 9] = 1.0

        # per-slot gather group indices (16-wrap layout) + 4 bucket masks
        gidx16 = np.zeros((16, 8 * NG), dtype=np.int16)
        cmask = np.zeros((128, 4, NG), dtype=bf16)
        for r in range(NUM_REL):
            d = per_core[c][r]
            if len(d["s"]) == 0:
                continue
            s = d["s"]; t = d["t"]; k = d["k"]
            cols = colbase_r[r][k] + s // 128
            p = s % 128
            cv = (1.0 / np.maximum(d["dcnt"][d["perm"]], 1.0)).astype(f32)
            gidx16[p % 16, 8 * cols + p // 16] = (t // 4).astype(np.int16)
            cmask[p, t % 4, cols] = cv[s].astype(bf16)
        # pack per piece: [8ck i16 idx | 4ck bf16 cmask]
        gconst = np.zeros((128, 12 * NG), dtype=np.int16)
        for r in range(NUM_REL):
            for (c0, ck, _) in cfg.pieces[r]:
                blk = np.tile(gidx16[:, 8 * c0:8 * (c0 + ck)], (8, 1))
                gconst[:, 12 * c0:12 * c0 + 8 * ck] = blk
                cm = np.ascontiguousarray(
                    cmask[:, :, c0:c0 + ck].reshape(128, 4 * ck)).view(np.int16)
                gconst[:, 12 * c0 + 8 * ck:12 * (c0 + ck)] = cm

        # unpermute gather indices: canonical slot n reads scratch[rank[n]]
        uidx = np.zeros((16, NUM_REL, 8 * CUNP), dtype=np.int16)
        n = np.arange(SP)
        for r in range(NUM_REL):
            d = per_core[c][r]
            uidx[n % 16, r, n // 16] = d["rank"][n].astype(np.int16)
        uidx = np.tile(uidx, (8, 1, 1))

        in_maps.append({
            "des": des, "tweet": tw, "numcat": ncat,
            "gconst": gconst, "uidx": uidx,
            "wd": wd, "wt": wt, "wnc": wnc, "wsm": wsm,
            "id128": id128, "id32": id32,
        })
    return cfg, in_maps


_CACHE = {}


def kernel(**inputs):
    cfg, in_maps = _prep(inputs)
    key = (tuple(sorted(ABL)),) + tuple(
        (c0, ck) for r in range(NUM_REL) for (c0, ck, _) in cfg.pieces[r])
    if key not in _CACHE:
        _CACHE[key] = build_bass(cfg)
    nc = _CACHE[key]
    res = run_bass_kernel_spmd(nc, in_maps, list(range(N_CORES)))
    outs = []
    for c in range(N_CORES):
        o = res.results[c]["out"]
        outs.append(o.T[0:cfg.shard_real])
    return np.ascontiguousarray(np.concatenate(outs, axis=0).astype(np.float32))
